# revision 36
# baseline (speedup 1.0000x reference)
"""Multi-head attention (B=2, S=T=2048, D=1024, H=16) on 8 TRN2 NeuronCores.

Sharding: 2-way data parallel over batch x 4-way tensor parallel over heads.
Core c handles batch c//4 and heads [4*(c%4), 4*(c%4)+4).

All matmuls run in bf16 with fp32 PSUM. fp8 was evaluated and rejected:
relative noise on attention weights or on v passes ~1:1 into the output
(the output is itself a weighted average, so averaging shrinks signal and
noise together) — e5m2 exp'd scores alone measure 6.2e-2 L2, over the
2e-2 gate. DoubleRow fp8 also only reaches ~263ns per 512-col instr on
hardware (vs 259ns bf16): matmul streaming is moving-operand-bandwidth
bound, so fp8 buys no tensor time either.

Each head's 64 v columns sit next to a 64-wide all-ones block, so the
PV matmul also emits the softmax denominator replicated across 64
partitions; normalization is a copy + reciprocal + multiply on the DVE.
The two QKT matmuls of a head pair use 64-partition stationaries in
disjoint PE row halves and execute concurrently (~387ns for both).

Schedule: only kT[m0] + qT[m0, first t-block] projections run before the
attention stream starts (first exp ~25us vs ~59us when all projections
run upfront). The remaining q/k projections, the v projection, and the
deferred out-projection chunks drip into the attention loop's spare
tensor slots. All PSUM drains run on the DVE so the ACT engine is
exp-only during the stream.
"""

import sys
import types

import numpy as np

import concourse.bass as bass  # noqa: F401  (registers engine classes)
import concourse.tile as tile
import concourse.mybir as mybir
from concourse import bacc
from concourse.bass import ts
from concourse.bass_utils import run_bass_kernel_spmd

FP32 = mybir.dt.float32
BF16 = mybir.dt.bfloat16
AF = mybir.ActivationFunctionType

D_MODEL = 1024
NUM_HEADS = 16
D_HEAD = 64
SCALING = D_HEAD ** -0.5
N_CORES = 8
DP = 2                      # data-parallel over batch
TPG = N_CORES // DP         # 4 tensor-parallel groups
DC = D_MODEL // TPG         # 256 output dims per core
HPC = DC // D_HEAD          # 4 heads per core

PV_PLAG = 2                 # PV lag in sc-pairs behind the exp stream

PROFILE = False             # set by test harness; collects exec_time_ns
LAST_EXEC_NS = None
LAST_RESULTS = None

_programs = {}


def _install_profile_hook():
    if "antenv.axon_hooks" in sys.modules:
        return
    try:
        from trn_agent_boot.trn_boot import _ntff_profile_via_ctypes
        hook = _ntff_profile_via_ctypes("/opt/axon/libaxon_pjrt.so")
    except Exception:
        hook = None
    mod = types.ModuleType("antenv.axon_hooks")
    mod.get_axon_ntff_profile_hook = lambda: hook
    mod.set_axon_ntff_profile_hook = lambda h: None
    sys.modules["antenv.axon_hooks"] = mod


def build_program(has_bias=False, has_mask=False, T=2048, S=2048, D=D_MODEL,
                  DCL=DC, TT=512):
    """Build the per-core bass program (SPMD: same program, per-core inputs)."""
    KC = D // 128            # contraction chunks
    SC = S // 128            # s chunks (PV contraction)
    NTT = T // TT            # t tiles
    MC = DCL // 128          # qT/kT partition chunks
    HP = (DCL // D_HEAD) // 2  # head pairs
    PT = 2 * TT              # projection tile width (psS-shaped)
    NET = D // TT            # out-proj e tiles

    nc = bacc.Bacc("TRN2", target_bir_lowering=False, debug=False)
    xq_t = nc.dram_tensor("xq_t", [D, T], BF16, kind="ExternalInput")
    xkv_t = nc.dram_tensor("xkv_t", [D, S], BF16, kind="ExternalInput")
    wq_t = nc.dram_tensor("wq_t", [D, DCL], BF16, kind="ExternalInput")
    wk_t = nc.dram_tensor("wk_t", [D, DCL], BF16, kind="ExternalInput")
    wv_t = nc.dram_tensor("wv_t", [D, DCL], BF16, kind="ExternalInput")
    wo_t = nc.dram_tensor("wo_t", [DCL, D], BF16, kind="ExternalInput")
    y_t = nc.dram_tensor("y", [T, D], BF16, kind="ExternalOutput")
    if has_bias:
        bq_t = nc.dram_tensor("bq_t", [DCL], FP32, kind="ExternalInput")
        bk_t = nc.dram_tensor("bk_t", [DCL], FP32, kind="ExternalInput")
        bv_t = nc.dram_tensor("bv_t", [DCL], FP32, kind="ExternalInput")
    if has_mask:
        mask_t = nc.dram_tensor("mask_t", [S, T], FP32, kind="ExternalInput")

    with tile.TileContext(nc) as tc:
        with tc.tile_pool(name="w", bufs=1) as wpool, \
             tc.tile_pool(name="big", bufs=1) as big, \
             tc.tile_pool(name="r", bufs=4) as rpool, \
             tc.tile_pool(name="yst", bufs=3) as ypool, \
             tc.tile_pool(name="psS", bufs=2, space="PSUM") as psS, \
             tc.tile_pool(name="psC", bufs=2, space="PSUM") as psC, \
             tc.tile_pool(name="psO", bufs=2, space="PSUM") as psO, \
             (tc.tile_pool(name="msk", bufs=4) if has_mask else _nullpool()) as mpool:

            # ---- persistent weights / activations ----
            wk_sb = wpool.tile([128, KC, DCL], BF16, tag="wk")
            wq_sb = wpool.tile([128, KC, DCL], BF16, tag="wq")
            wv_sb = wpool.tile([128, KC, DCL], BF16, tag="wv")
            wo_sb = wpool.tile([128, MC, D], BF16, tag="wo")
            xkv_sb = big.tile([128, KC, S], BF16, tag="xkv")
            xq_sb = big.tile([128, KC, T], BF16, tag="xq")
            kT_sb = big.tile([128, MC, S], BF16, tag="kT")
            qT_sb = big.tile([128, MC, T], BF16, tag="qT")
            ct_sb = big.tile([128, MC, T], BF16, tag="ct")
            # v blocks: [head0 d64 | ones64 | head1 d64 | ones64 | ...]
            v_sb = big.tile([128, SC, 2 * HPC, D_HEAD], BF16, tag="v")
            # exp'd scores ring, one slot per key chunk of the live group
            e_ring = big.tile([128, SC, 2 * TT], BF16, tag="er")

            ones_col_f = wpool.tile([128, 1], FP32, tag="onescolf")
            nc.gpsimd.memset(ones_col_f[:], 1.0)
            ones_col = wpool.tile([128, 1], BF16, tag="onescol")
            nc.vector.tensor_copy(ones_col[:], ones_col_f[:])

            # ---- PE warm-up: the tensor engine reaches full clock only
            # after ~3us of continuous execution. Dummy matmuls during the
            # input-DMA window ramp it so the real projections run at speed.
            warm = wpool.tile([128, 512], BF16, tag="warm")
            nc.gpsimd.memset(warm[:], 0.25)
            for _ in range(9):
                pw = psS.tile([128, PT], FP32, tag="s")
                nc.tensor.matmul(pw[:, 0:TT], warm[:, 0:128], warm[:],
                                 start=True, stop=True)
                nc.tensor.matmul(pw[:, TT:2 * TT], warm[:, 0:128], warm[:],
                                 start=True, stop=True)
            # v viewed as [p, sc, head, (data|ones), c]
            v5 = v_sb[:].rearrange("p s (h two) c -> p s h two c", two=2)
            nc.vector.tensor_copy(
                v5[:, :, :, 1, :],
                ones_col[:].to_broadcast((128, SC, HPC, 1, D_HEAD)),
            )

            # ---- input DMAs ----
            # Queues run at ~30-35 GB/s each, so a 512KB weight on one queue
            # gates the first projection by ~15us. The first wave spreads the
            # critical bytes (wk pieces, the first xkv column-half, wq
            # pieces) across all 16 queues; later waves follow first use.
            def dma_w(w_sb, w_t, pieces=1):
                for p in range(pieces):
                    kc = KC // pieces
                    nc.sync.dma_start(
                        w_sb[:, p * kc:(p + 1) * kc, :],
                        w_t.ap().rearrange("(c p) d -> p c d", p=128)
                        [:, p * kc:(p + 1) * kc, :])

            # wave 1 — 16 equal ~128KB pieces, one per queue: wk + the xkv
            # columns of the first kT s-block + wq
            dma_w(wk_sb, wk_t, pieces=4)
            for c in range(KC):
                nc.sync.dma_start(
                    xkv_sb[:, c, 0:TT],
                    xkv_t.ap()[c * 128:(c + 1) * 128, 0:TT])
            dma_w(wq_sb, wq_t, pieces=4)
            # wave 2 — the upfront qT block's xq columns, the second kT
            # s-block, wv for the dripped v-projection
            for c in range(KC):
                nc.sync.dma_start(
                    xq_sb[:, c, 0:TT], xq_t.ap()[c * 128:(c + 1) * 128, 0:TT])
            for c in range(KC):
                nc.sync.dma_start(
                    xkv_sb[:, c, TT:2 * TT],
                    xkv_t.ap()[c * 128:(c + 1) * 128, TT:2 * TT])
            dma_w(wv_sb, wv_t, pieces=4)
            # wave 3 — the rest
            for c in range(KC):
                nc.sync.dma_start(
                    xkv_sb[:, c, 2 * TT:S],
                    xkv_t.ap()[c * 128:(c + 1) * 128, 2 * TT:S])
            for c in range(KC):
                nc.sync.dma_start(
                    xq_sb[:, c, TT:T], xq_t.ap()[c * 128:(c + 1) * 128, TT:T])
            if has_bias:
                bq_sb = wpool.tile([128, MC], FP32, tag="bq")
                nc.sync.dma_start(bq_sb[:], bq_t.ap().rearrange("(m p) -> p m", p=128))
                bk_sb = wpool.tile([128, MC], FP32, tag="bk")
                nc.sync.dma_start(bk_sb[:], bk_t.ap().rearrange("(m p) -> p m", p=128))
                bv_sb = wpool.tile([128, MC], FP32, tag="bv")
                nc.sync.dma_start(bv_sb[:], bv_t.ap().rearrange("(m p) -> p m", p=128))
            nc.sync.dma_start(
                wo_sb[:], wo_t.ap().rearrange("(m p) e -> p m e", p=128))

            # ---- kT/qT projections (1024-wide PSUM tiles, DVE drains) ----
            def proj(dst_m, w_sb_, x_sb, m, hh, pool, bias=None):
                # one 512-wide block (the matmul ISA caps the moving dim at
                # 512). Upfront blocks go through psS; blocks dripped into
                # the attention loop use psO so they never block the score
                # tiles' 2-slot psS rotation (which would stall the exps).
                ps = pool.tile([128, TT], FP32,
                               tag="s" if pool is psS else "o")
                for c in range(KC):
                    nc.tensor.matmul(
                        ps[:], w_sb_[:, c, ts(m, 128)],
                        x_sb[:, c, ts(hh, TT)],
                        start=(c == 0), stop=(c == KC - 1))
                dst = dst_m[:, ts(hh, TT)]
                if bias is not None:
                    nc.vector.tensor_scalar_add(dst, ps[:], bias)
                else:
                    nc.vector.tensor_copy(dst, ps[:])

            def emit_proj(which, m, hh, pool=psS):
                if which == 0:
                    proj(kT_sb[:, m, :], wk_sb, xkv_sb, m, hh, pool,
                         bk_sb[:, m:m + 1] if has_bias else None)
                else:
                    proj(qT_sb[:, m, :], wq_sb, xq_sb, m, hh, pool,
                         bq_sb[:, m:m + 1] if has_bias else None)

            # upfront (dense, full-clock after warmup): just the first
            # s-block of kT[m0] and t-block of qT[m0]. kT's later s-blocks
            # (first used at sc 4/8/12) drip through the then-idle psO pool
            # in the first iterations; everything else drips in first-use
            # order (kT/qT m1 at group (0,1) = iter 16+, qT later t-blocks
            # at t-tiles 1-3).
            emit_proj(0, 0, 0)
            emit_proj(0, 0, 1)
            emit_proj(1, 0, 0)
            early_proj = [(0, 0, 2), (0, 0, 3)]
            pending_proj = [(0, 1, 0), (0, 1, 1), (1, 1, 0), (0, 1, 2),
                            (0, 1, 3), (1, 0, 1), (1, 1, 1), (1, 0, 2),
                            (1, 1, 2), (1, 0, 3), (1, 1, 3)]

            # ---- v projection chunk (dripped into the first group) ----
            def emit_vproj(sc):
                ps = psO.tile([128, TT], FP32, tag="o")
                psv = ps[:, 0:DCL]
                last = KC - 1
                for c in range(KC):
                    nc.tensor.matmul(
                        psv, xkv_sb[:, c, ts(sc, 128)], wv_sb[:, c, :],
                        start=(c == 0), stop=(c == last and not has_bias))
                pshc = psv.rearrange("p (h c) -> p h c", c=D_HEAD)
                if has_bias:
                    # bias via DVE add during the copy
                    nc.vector.tensor_scalar_add(
                        v5[:, sc, :, 0, :], pshc, bv_sb[:, 0:1])
                else:
                    nc.vector.tensor_copy(v5[:, sc, :, 0, :], pshc)

            # ---- out-projection chunks (deferred, dripped) ----
            def emit_outproj_chunk(tq, et, on_act=False, pool=None):
                pool = psO if pool is None else pool
                ysb = ypool.tile([128, TT], BF16, tag="y", name=f"ysb{tq}_{et}")
                ps = pool.tile([128, TT], FP32,
                               tag="s" if pool is psS else "o")
                for m in range(MC):
                    nc.tensor.matmul(
                        ps[:], ct_sb[:, m, ts(tq, 128)], wo_sb[:, m, ts(et, TT)],
                        start=(m == 0), stop=(m == MC - 1))
                if on_act:
                    nc.scalar.activation(ysb[:], ps[:], AF.Copy)
                else:
                    nc.vector.tensor_copy(ysb[:], ps[:])
                # two 64-row pieces land on two queues (the final chunks'
                # writeback is on the critical path at the tail)
                nc.sync.dma_start(
                    y_t.ap()[tq * 128:tq * 128 + 64, ts(et, TT)], ysb[0:64, :])
                nc.sync.dma_start(
                    y_t.ap()[tq * 128 + 64:tq * 128 + 128, ts(et, TT)],
                    ysb[64:128, :])

            # ---- attention: QKT/exp stream + lagged PV queue ----
            ctx = {}            # (tt, hp) -> [cA, cB]

            def emit_den(cps, lh, tt, hp):
                # copy numerators out too so the PSUM slot frees after two
                # short copies free the PSUM slot for the next group's PV
                den = rpool.tile([64, TT], FP32, tag="den")
                nc.vector.tensor_copy(den[:], cps[64:128, :])
                num = rpool.tile([64, TT], FP32, tag="num")
                nc.vector.tensor_copy(num[:], cps[0:64, :])
                rec = rpool.tile([64, TT], FP32, tag="rec")
                nc.vector.reciprocal_approx_fast(rec[:], den[:])
                dst = ct_sb[lh:lh + 64, hp, ts(tt, TT)]
                nc.vector.tensor_mul(dst, num[:], rec[:])

            def pop_pv(pvq):
                tt, hp, sc = pvq.pop(0)
                ha, hb = 2 * hp, 2 * hp + 1
                if sc == 0:
                    ctx[(tt, hp)] = [
                        psC.tile([128, TT], FP32, tag="c", name=f"cA{tt}_{hp}"),
                        psC.tile([128, TT], FP32, tag="c", name=f"cB{tt}_{hp}"),
                    ]
                cA, cB = ctx[(tt, hp)]
                start, stop = (sc == 0), (sc == SC - 1)
                nc.tensor.matmul(
                    cA[:], v_sb[:, sc, 2 * ha:2 * ha + 2, :],
                    e_ring[:, sc, 0:TT], start=start, stop=stop)
                nc.tensor.matmul(
                    cB[:], v_sb[:, sc, 2 * hb:2 * hb + 2, :],
                    e_ring[:, sc, TT:2 * TT], start=start, stop=stop)
                if stop:
                    emit_den(cA, 0, tt, hp)
                    emit_den(cB, 64, tt, hp)
                    del ctx[(tt, hp)]

            pvq = []
            deferred = []
            for tt in range(NTT):
                for hp in range(HP):
                    for sc in range(SC):
                        it = hp * SC + sc
                        sAB = psS.tile([128, 2 * TT], FP32, tag="s")
                        nc.tensor.matmul(
                            sAB[:, 0:TT], kT_sb[0:64, hp, ts(sc, 128)],
                            qT_sb[0:64, hp, ts(tt, TT)], start=True, stop=True)
                        nc.tensor.matmul(
                            sAB[:, TT:2 * TT], kT_sb[64:128, hp, ts(sc, 128)],
                            qT_sb[64:128, hp, ts(tt, TT)], start=True, stop=True)
                        if has_mask:
                            mt = mpool.tile([128, TT], FP32, tag="m")
                            nc.sync.dma_start(
                                mt[:], mask_t.ap()[ts(sc, 128), ts(tt, TT)])
                            nc.vector.tensor_add(sAB[:, 0:TT], sAB[:, 0:TT], mt[:])
                            nc.vector.tensor_add(sAB[:, TT:2 * TT],
                                                 sAB[:, TT:2 * TT], mt[:])
                        nc.scalar.activation(e_ring[:, sc, :], sAB[:], AF.Exp)
                        pvq.append((tt, hp, sc))
                        # fillers behind this iteration's QKT: v-projection
                        # chunks (shifted 3 late so the first exps aren't
                        # tensor-bound), the remaining kT/qT blocks, the
                        # deferred out-projection chunks.
                        if tt == 0 and it < 3 and early_proj:
                            emit_proj(*early_proj.pop(0), pool=psO)
                        if tt == 0 and 3 <= it < 3 + SC:
                            emit_vproj(it - 3)
                        if tt == 0 and sc % 2 == 0 and pending_proj:
                            emit_proj(*pending_proj.pop(0))
                        # ct of t-tile tt-1 is final only a few iterations
                        # into (tt, hp0) — keep hp0 drains late, hp1 early.
                        drain = (sc % 2 == 1) and \
                            (sc >= 7 if hp == 0 else sc <= 7)
                        if deferred and drain:
                            emit_outproj_chunk(*deferred.pop(0))
                        if len(pvq) > 2 * PV_PLAG:
                            pop_pv(pvq)
                # queue this t-tile's output projection
                for tq in range(tt * (TT // 128), (tt + 1) * (TT // 128)):
                    for et in range(NET):
                        deferred.append((tq, et))
            while pvq:
                pop_pv(pvq)
            # tail out-projections: psS is free once the exps are done, so
            # alternate psO/psS for a 4-deep chunk pipeline
            for i, (tq, et) in enumerate(deferred):
                emit_outproj_chunk(tq, et, on_act=(i % 2 == 0),
                                   pool=(psS if i % 2 else psO))

    nc.compile()
    return nc


class _nullpool:
    def __enter__(self):
        return None

    def __exit__(self, *a):
        return False


def _get_program(has_bias, has_mask):
    key = (has_bias, has_mask)
    if key not in _programs:
        _programs[key] = build_program(has_bias, has_mask)
    return _programs[key]


def kernel(query_states, key_value_states, attention_mask,
           Wq, bq, Wk, bk, Wv, bv, Wo, bo):
    global LAST_EXEC_NS, LAST_RESULTS
    import ml_dtypes
    bf16 = ml_dtypes.bfloat16
    q = np.asarray(query_states, dtype=np.float32)
    kv = np.asarray(key_value_states, dtype=np.float32)
    mask = np.asarray(attention_mask, dtype=np.float32)
    Wq = np.asarray(Wq, np.float32); bq = np.asarray(bq, np.float32)
    Wk = np.asarray(Wk, np.float32); bk = np.asarray(bk, np.float32)
    Wv = np.asarray(Wv, np.float32); bv = np.asarray(bv, np.float32)
    Wo = np.asarray(Wo, np.float32); bo = np.asarray(bo, np.float32)

    has_bias = bool(np.any(bq) or np.any(bk) or np.any(bv))
    has_mask = bool(np.any(mask))
    nc = _get_program(has_bias, has_mask)

    # per-batch activations (shared across the 4 TP cores of each batch)
    xq16 = [np.ascontiguousarray(q[b].T.astype(bf16)) for b in range(DP)]
    xkv16 = [np.ascontiguousarray(kv[b].T.astype(bf16)) for b in range(DP)]

    in_maps = []
    for c in range(N_CORES):
        b, hg = divmod(c, TPG)
        sl = slice(DC * hg, DC * (hg + 1))
        m = {
            "xq_t": xq16[b],
            "xkv_t": xkv16[b],
            "wq_t": np.ascontiguousarray((Wq[sl] * SCALING).T.astype(bf16)),
            "wk_t": np.ascontiguousarray(Wk[sl].T.astype(bf16)),
            "wv_t": np.ascontiguousarray(Wv[sl].T.astype(bf16)),
            "wo_t": np.ascontiguousarray(Wo[:, sl].T.astype(bf16)),
        }
        if has_bias:
            m["bq_t"] = np.ascontiguousarray(bq[sl] * SCALING)
            m["bk_t"] = np.ascontiguousarray(bk[sl])
            m["bv_t"] = np.ascontiguousarray(bv[sl])
        if has_mask:
            mb = np.broadcast_to(mask[b].reshape(-1, mask.shape[-2], mask.shape[-1])[0],
                                 (q.shape[1], kv.shape[1]))
            m["mask_t"] = np.ascontiguousarray(mb.T)
        in_maps.append(m)

    if PROFILE:
        _install_profile_hook()
    res = run_bass_kernel_spmd(nc, in_maps, core_ids=list(range(N_CORES)),
                               trace=bool(PROFILE))
    LAST_EXEC_NS = res.exec_time_ns
    LAST_RESULTS = res
    outs = [res.results[c]["y"].astype(np.float32) for c in range(N_CORES)]
    y = np.stack([sum(outs[b * TPG:(b + 1) * TPG]) for b in range(DP)])
    return (y + bo).astype(np.float32)


# revision 37
# speedup vs baseline: 1.0059x; 1.0059x over previous
"""Multi-head attention (B=2, S=T=2048, D=1024, H=16) on 8 TRN2 NeuronCores.

Sharding: 2-way data parallel over batch x 4-way tensor parallel over heads.
Core c handles batch c//4 and heads [4*(c%4), 4*(c%4)+4).

All matmuls run in bf16 with fp32 PSUM. fp8 was evaluated and rejected:
relative noise on attention weights or on v passes ~1:1 into the output
(the output is itself a weighted average, so averaging shrinks signal and
noise together) — e5m2 exp'd scores alone measure 6.2e-2 L2, over the
2e-2 gate. DoubleRow fp8 also only reaches ~263ns per 512-col instr on
hardware (vs 259ns bf16): matmul streaming is moving-operand-bandwidth
bound, so fp8 buys no tensor time either.

Each head's 64 v columns sit next to a 64-wide all-ones block, so the
PV matmul also emits the softmax denominator replicated across 64
partitions; normalization is a copy + reciprocal + multiply on the DVE.
The two QKT matmuls of a head pair use 64-partition stationaries in
disjoint PE row halves and execute concurrently (~387ns for both).

Schedule: only kT[m0] + qT[m0, first t-block] projections run before the
attention stream starts (first exp ~25us vs ~59us when all projections
run upfront). The remaining q/k projections, the v projection, and the
deferred out-projection chunks drip into the attention loop's spare
tensor slots. All PSUM drains run on the DVE so the ACT engine is
exp-only during the stream.
"""

import sys
import types

import numpy as np

import concourse.bass as bass  # noqa: F401  (registers engine classes)
import concourse.tile as tile
import concourse.mybir as mybir
from concourse import bacc
from concourse.bass import ts
from concourse.bass_utils import run_bass_kernel_spmd

FP32 = mybir.dt.float32
BF16 = mybir.dt.bfloat16
AF = mybir.ActivationFunctionType

D_MODEL = 1024
NUM_HEADS = 16
D_HEAD = 64
SCALING = D_HEAD ** -0.5
N_CORES = 8
DP = 2                      # data-parallel over batch
TPG = N_CORES // DP         # 4 tensor-parallel groups
DC = D_MODEL // TPG         # 256 output dims per core
HPC = DC // D_HEAD          # 4 heads per core

PV_PLAG = 2                 # PV lag in sc-pairs behind the exp stream

PROFILE = False             # set by test harness; collects exec_time_ns
LAST_EXEC_NS = None
LAST_RESULTS = None

_programs = {}


def _install_profile_hook():
    if "antenv.axon_hooks" in sys.modules:
        return
    try:
        from trn_agent_boot.trn_boot import _ntff_profile_via_ctypes
        hook = _ntff_profile_via_ctypes("/opt/axon/libaxon_pjrt.so")
    except Exception:
        hook = None
    mod = types.ModuleType("antenv.axon_hooks")
    mod.get_axon_ntff_profile_hook = lambda: hook
    mod.set_axon_ntff_profile_hook = lambda h: None
    sys.modules["antenv.axon_hooks"] = mod


def build_program(has_bias=False, has_mask=False, T=2048, S=2048, D=D_MODEL,
                  DCL=DC, TT=512):
    """Build the per-core bass program (SPMD: same program, per-core inputs)."""
    KC = D // 128            # contraction chunks
    SC = S // 128            # s chunks (PV contraction)
    NTT = T // TT            # t tiles
    MC = DCL // 128          # qT/kT partition chunks
    HP = (DCL // D_HEAD) // 2  # head pairs
    PT = 2 * TT              # projection tile width (psS-shaped)
    NET = D // TT            # out-proj e tiles

    nc = bacc.Bacc("TRN2", target_bir_lowering=False, debug=False)
    xq_t = nc.dram_tensor("xq_t", [D, T], BF16, kind="ExternalInput")
    xkv_t = nc.dram_tensor("xkv_t", [D, S], BF16, kind="ExternalInput")
    wq_t = nc.dram_tensor("wq_t", [D, DCL], BF16, kind="ExternalInput")
    wk_t = nc.dram_tensor("wk_t", [D, DCL], BF16, kind="ExternalInput")
    wv_t = nc.dram_tensor("wv_t", [D, DCL], BF16, kind="ExternalInput")
    wo_t = nc.dram_tensor("wo_t", [DCL, D], BF16, kind="ExternalInput")
    y_t = nc.dram_tensor("y", [T, D], BF16, kind="ExternalOutput")
    if has_bias:
        bq_t = nc.dram_tensor("bq_t", [DCL], FP32, kind="ExternalInput")
        bk_t = nc.dram_tensor("bk_t", [DCL], FP32, kind="ExternalInput")
        bv_t = nc.dram_tensor("bv_t", [DCL], FP32, kind="ExternalInput")
    if has_mask:
        mask_t = nc.dram_tensor("mask_t", [S, T], FP32, kind="ExternalInput")

    with tile.TileContext(nc) as tc:
        with tc.tile_pool(name="w", bufs=1) as wpool, \
             tc.tile_pool(name="big", bufs=1) as big, \
             tc.tile_pool(name="r", bufs=4) as rpool, \
             tc.tile_pool(name="yst", bufs=3) as ypool, \
             tc.tile_pool(name="psS", bufs=2, space="PSUM") as psS, \
             tc.tile_pool(name="psC", bufs=2, space="PSUM") as psC, \
             tc.tile_pool(name="psO", bufs=2, space="PSUM") as psO, \
             (tc.tile_pool(name="msk", bufs=4) if has_mask else _nullpool()) as mpool:

            # ---- persistent weights / activations ----
            wk_sb = wpool.tile([128, KC, DCL], BF16, tag="wk")
            wq_sb = wpool.tile([128, KC, DCL], BF16, tag="wq")
            wv_sb = wpool.tile([128, KC, DCL], BF16, tag="wv")
            wo_sb = wpool.tile([128, MC, D], BF16, tag="wo")
            xkv_sb = big.tile([128, KC, S], BF16, tag="xkv")
            xq_sb = big.tile([128, KC, T], BF16, tag="xq")
            kT_sb = big.tile([128, MC, S], BF16, tag="kT")
            qT_sb = big.tile([128, MC, T], BF16, tag="qT")
            ct_sb = big.tile([128, MC, T], BF16, tag="ct")
            # v blocks: [head0 d64 | ones64 | head1 d64 | ones64 | ...]
            v_sb = big.tile([128, SC, 2 * HPC, D_HEAD], BF16, tag="v")
            # exp'd scores ring, one slot per key chunk of the live group
            e_ring = big.tile([128, SC, 2 * TT], BF16, tag="er")

            ones_col_f = wpool.tile([128, 1], FP32, tag="onescolf")
            nc.gpsimd.memset(ones_col_f[:], 1.0)
            ones_col = wpool.tile([128, 1], BF16, tag="onescol")
            nc.vector.tensor_copy(ones_col[:], ones_col_f[:])

            # ---- PE warm-up: the tensor engine reaches full clock only
            # after ~3us of continuous execution. Dummy matmuls during the
            # input-DMA window ramp it so the real projections run at speed.
            warm = wpool.tile([128, 512], BF16, tag="warm")
            nc.gpsimd.memset(warm[:], 0.25)
            for _ in range(9):
                pw = psS.tile([128, PT], FP32, tag="s")
                nc.tensor.matmul(pw[:, 0:TT], warm[:, 0:128], warm[:],
                                 start=True, stop=True)
                nc.tensor.matmul(pw[:, TT:2 * TT], warm[:, 0:128], warm[:],
                                 start=True, stop=True)
            # v viewed as [p, sc, head, (data|ones), c]
            v5 = v_sb[:].rearrange("p s (h two) c -> p s h two c", two=2)
            nc.vector.tensor_copy(
                v5[:, :, :, 1, :],
                ones_col[:].to_broadcast((128, SC, HPC, 1, D_HEAD)),
            )

            # ---- input DMAs ----
            # Queues run at ~30-35 GB/s each, so a 512KB weight on one queue
            # gates the first projection by ~15us. The first wave spreads the
            # critical bytes (wk pieces, the first xkv column-half, wq
            # pieces) across all 16 queues; later waves follow first use.
            def dma_w(w_sb, w_t, pieces=1):
                for p in range(pieces):
                    kc = KC // pieces
                    nc.sync.dma_start(
                        w_sb[:, p * kc:(p + 1) * kc, :],
                        w_t.ap().rearrange("(c p) d -> p c d", p=128)
                        [:, p * kc:(p + 1) * kc, :])

            # wave 1 — 16 equal ~128KB pieces, one per queue: wk + the xkv
            # columns of the first kT s-block + wq
            dma_w(wk_sb, wk_t, pieces=4)
            for c in range(KC):
                nc.sync.dma_start(
                    xkv_sb[:, c, 0:TT],
                    xkv_t.ap()[c * 128:(c + 1) * 128, 0:TT])
            dma_w(wq_sb, wq_t, pieces=4)
            # wave 2 — the upfront qT block's xq columns, the second kT
            # s-block, wv for the dripped v-projection
            for c in range(KC):
                nc.sync.dma_start(
                    xq_sb[:, c, 0:TT], xq_t.ap()[c * 128:(c + 1) * 128, 0:TT])
            for c in range(KC):
                nc.sync.dma_start(
                    xkv_sb[:, c, TT:2 * TT],
                    xkv_t.ap()[c * 128:(c + 1) * 128, TT:2 * TT])
            dma_w(wv_sb, wv_t, pieces=4)
            # wave 3 — the rest
            for c in range(KC):
                nc.sync.dma_start(
                    xkv_sb[:, c, 2 * TT:S],
                    xkv_t.ap()[c * 128:(c + 1) * 128, 2 * TT:S])
            for c in range(KC):
                nc.sync.dma_start(
                    xq_sb[:, c, TT:T], xq_t.ap()[c * 128:(c + 1) * 128, TT:T])
            if has_bias:
                bq_sb = wpool.tile([128, MC], FP32, tag="bq")
                nc.sync.dma_start(bq_sb[:], bq_t.ap().rearrange("(m p) -> p m", p=128))
                bk_sb = wpool.tile([128, MC], FP32, tag="bk")
                nc.sync.dma_start(bk_sb[:], bk_t.ap().rearrange("(m p) -> p m", p=128))
                bv_sb = wpool.tile([128, MC], FP32, tag="bv")
                nc.sync.dma_start(bv_sb[:], bv_t.ap().rearrange("(m p) -> p m", p=128))
            nc.sync.dma_start(
                wo_sb[:], wo_t.ap().rearrange("(m p) e -> p m e", p=128))

            # ---- kT/qT projections (1024-wide PSUM tiles, DVE drains) ----
            def proj(dst_m, w_sb_, x_sb, m, hh, pool, bias=None):
                # one 512-wide block (the matmul ISA caps the moving dim at
                # 512). Upfront blocks go through psS; blocks dripped into
                # the attention loop use psO so they never block the score
                # tiles' 2-slot psS rotation (which would stall the exps).
                ps = pool.tile([128, TT], FP32,
                               tag="s" if pool is psS else "o")
                for c in range(KC):
                    nc.tensor.matmul(
                        ps[:], w_sb_[:, c, ts(m, 128)],
                        x_sb[:, c, ts(hh, TT)],
                        start=(c == 0), stop=(c == KC - 1))
                dst = dst_m[:, ts(hh, TT)]
                if bias is not None:
                    nc.vector.tensor_scalar_add(dst, ps[:], bias)
                else:
                    nc.vector.tensor_copy(dst, ps[:])

            def emit_proj(which, m, hh, pool=psS):
                if which == 0:
                    proj(kT_sb[:, m, :], wk_sb, xkv_sb, m, hh, pool,
                         bk_sb[:, m:m + 1] if has_bias else None)
                else:
                    proj(qT_sb[:, m, :], wq_sb, xq_sb, m, hh, pool,
                         bq_sb[:, m:m + 1] if has_bias else None)

            # upfront (dense, full-clock after warmup): just the first
            # s-block of kT[m0] and t-block of qT[m0]. kT's later s-blocks
            # (first used at sc 4/8/12) drip through the then-idle psO pool
            # in the first iterations; everything else drips in first-use
            # order (kT/qT m1 at group (0,1) = iter 16+, qT later t-blocks
            # at t-tiles 1-3).
            emit_proj(0, 0, 0)
            emit_proj(0, 0, 1)
            emit_proj(1, 0, 0)
            early_proj = [(0, 0, 2), (0, 0, 3)]
            pending_proj = [(0, 1, 0), (0, 1, 1), (1, 1, 0), (0, 1, 2),
                            (0, 1, 3), (1, 0, 1), (1, 1, 1), (1, 0, 2),
                            (1, 1, 2), (1, 0, 3), (1, 1, 3)]

            # ---- v projection chunk (dripped into the first group) ----
            def emit_vproj(sc):
                ps = psO.tile([128, TT], FP32, tag="o")
                psv = ps[:, 0:DCL]
                last = KC - 1
                for c in range(KC):
                    nc.tensor.matmul(
                        psv, xkv_sb[:, c, ts(sc, 128)], wv_sb[:, c, :],
                        start=(c == 0), stop=(c == last and not has_bias))
                pshc = psv.rearrange("p (h c) -> p h c", c=D_HEAD)
                if has_bias:
                    # bias via DVE add during the copy
                    nc.vector.tensor_scalar_add(
                        v5[:, sc, :, 0, :], pshc, bv_sb[:, 0:1])
                else:
                    nc.vector.tensor_copy(v5[:, sc, :, 0, :], pshc)

            # ---- out-projection chunks (deferred, dripped) ----
            def emit_outproj_chunk(tq, et, on_act=False, pool=None):
                pool = psO if pool is None else pool
                ysb = ypool.tile([128, TT], BF16, tag="y", name=f"ysb{tq}_{et}")
                ps = pool.tile([128, TT], FP32,
                               tag="s" if pool is psS else "o")
                for m in range(MC):
                    nc.tensor.matmul(
                        ps[:], ct_sb[:, m, ts(tq, 128)], wo_sb[:, m, ts(et, TT)],
                        start=(m == 0), stop=(m == MC - 1))
                if on_act:
                    nc.scalar.activation(ysb[:], ps[:], AF.Copy)
                else:
                    nc.vector.tensor_copy(ysb[:], ps[:])
                nc.sync.dma_start(y_t.ap()[ts(tq, 128), ts(et, TT)], ysb[:])

            # ---- attention: QKT/exp stream + lagged PV queue ----
            ctx = {}            # (tt, hp) -> [cA, cB]

            def emit_den(cps, lh, tt, hp):
                # copy numerators out too so the PSUM slot frees after two
                # short copies free the PSUM slot for the next group's PV
                den = rpool.tile([64, TT], FP32, tag="den")
                nc.vector.tensor_copy(den[:], cps[64:128, :])
                num = rpool.tile([64, TT], FP32, tag="num")
                nc.vector.tensor_copy(num[:], cps[0:64, :])
                rec = rpool.tile([64, TT], FP32, tag="rec")
                nc.vector.reciprocal_approx_fast(rec[:], den[:])
                dst = ct_sb[lh:lh + 64, hp, ts(tt, TT)]
                nc.vector.tensor_mul(dst, num[:], rec[:])

            def pop_pv(pvq):
                tt, hp, sc = pvq.pop(0)
                ha, hb = 2 * hp, 2 * hp + 1
                if sc == 0:
                    ctx[(tt, hp)] = [
                        psC.tile([128, TT], FP32, tag="c", name=f"cA{tt}_{hp}"),
                        psC.tile([128, TT], FP32, tag="c", name=f"cB{tt}_{hp}"),
                    ]
                cA, cB = ctx[(tt, hp)]
                start, stop = (sc == 0), (sc == SC - 1)
                nc.tensor.matmul(
                    cA[:], v_sb[:, sc, 2 * ha:2 * ha + 2, :],
                    e_ring[:, sc, 0:TT], start=start, stop=stop)
                nc.tensor.matmul(
                    cB[:], v_sb[:, sc, 2 * hb:2 * hb + 2, :],
                    e_ring[:, sc, TT:2 * TT], start=start, stop=stop)
                if stop:
                    emit_den(cA, 0, tt, hp)
                    emit_den(cB, 64, tt, hp)
                    del ctx[(tt, hp)]

            pvq = []
            deferred = []
            for tt in range(NTT):
                for hp in range(HP):
                    for sc in range(SC):
                        it = hp * SC + sc
                        sAB = psS.tile([128, 2 * TT], FP32, tag="s")
                        nc.tensor.matmul(
                            sAB[:, 0:TT], kT_sb[0:64, hp, ts(sc, 128)],
                            qT_sb[0:64, hp, ts(tt, TT)], start=True, stop=True)
                        nc.tensor.matmul(
                            sAB[:, TT:2 * TT], kT_sb[64:128, hp, ts(sc, 128)],
                            qT_sb[64:128, hp, ts(tt, TT)], start=True, stop=True)
                        if has_mask:
                            mt = mpool.tile([128, TT], FP32, tag="m")
                            nc.sync.dma_start(
                                mt[:], mask_t.ap()[ts(sc, 128), ts(tt, TT)])
                            nc.vector.tensor_add(sAB[:, 0:TT], sAB[:, 0:TT], mt[:])
                            nc.vector.tensor_add(sAB[:, TT:2 * TT],
                                                 sAB[:, TT:2 * TT], mt[:])
                        nc.scalar.activation(e_ring[:, sc, :], sAB[:], AF.Exp)
                        pvq.append((tt, hp, sc))
                        # fillers behind this iteration's QKT: v-projection
                        # chunks (shifted 3 late so the first exps aren't
                        # tensor-bound), the remaining kT/qT blocks, the
                        # deferred out-projection chunks.
                        if tt == 0 and it < 3 and early_proj:
                            emit_proj(*early_proj.pop(0), pool=psO)
                        if tt == 0 and 3 <= it < 3 + SC:
                            emit_vproj(it - 3)
                        if tt == 0 and sc % 2 == 0 and pending_proj:
                            emit_proj(*pending_proj.pop(0))
                        # ct of t-tile tt-1 is final only a few iterations
                        # into (tt, hp0) — keep hp0 drains late, hp1 early.
                        drain = (sc % 2 == 1) and \
                            (sc >= 7 if hp == 0 else sc <= 7)
                        if deferred and drain:
                            emit_outproj_chunk(*deferred.pop(0))
                        if len(pvq) > 2 * PV_PLAG:
                            pop_pv(pvq)
                # queue this t-tile's output projection
                for tq in range(tt * (TT // 128), (tt + 1) * (TT // 128)):
                    for et in range(NET):
                        deferred.append((tq, et))
            while pvq:
                pop_pv(pvq)
            # tail out-projections: psS is free once the exps are done, so
            # alternate psO/psS for a 4-deep chunk pipeline
            for i, (tq, et) in enumerate(deferred):
                emit_outproj_chunk(tq, et, on_act=(i % 2 == 0),
                                   pool=(psS if i % 2 else psO))

    nc.compile()
    return nc


class _nullpool:
    def __enter__(self):
        return None

    def __exit__(self, *a):
        return False


def _get_program(has_bias, has_mask):
    key = (has_bias, has_mask)
    if key not in _programs:
        _programs[key] = build_program(has_bias, has_mask)
    return _programs[key]


def kernel(query_states, key_value_states, attention_mask,
           Wq, bq, Wk, bk, Wv, bv, Wo, bo):
    global LAST_EXEC_NS, LAST_RESULTS
    import ml_dtypes
    bf16 = ml_dtypes.bfloat16
    q = np.asarray(query_states, dtype=np.float32)
    kv = np.asarray(key_value_states, dtype=np.float32)
    mask = np.asarray(attention_mask, dtype=np.float32)
    Wq = np.asarray(Wq, np.float32); bq = np.asarray(bq, np.float32)
    Wk = np.asarray(Wk, np.float32); bk = np.asarray(bk, np.float32)
    Wv = np.asarray(Wv, np.float32); bv = np.asarray(bv, np.float32)
    Wo = np.asarray(Wo, np.float32); bo = np.asarray(bo, np.float32)

    has_bias = bool(np.any(bq) or np.any(bk) or np.any(bv))
    has_mask = bool(np.any(mask))
    nc = _get_program(has_bias, has_mask)

    # per-batch activations (shared across the 4 TP cores of each batch)
    xq16 = [np.ascontiguousarray(q[b].T.astype(bf16)) for b in range(DP)]
    xkv16 = [np.ascontiguousarray(kv[b].T.astype(bf16)) for b in range(DP)]

    in_maps = []
    for c in range(N_CORES):
        b, hg = divmod(c, TPG)
        sl = slice(DC * hg, DC * (hg + 1))
        m = {
            "xq_t": xq16[b],
            "xkv_t": xkv16[b],
            "wq_t": np.ascontiguousarray((Wq[sl] * SCALING).T.astype(bf16)),
            "wk_t": np.ascontiguousarray(Wk[sl].T.astype(bf16)),
            "wv_t": np.ascontiguousarray(Wv[sl].T.astype(bf16)),
            "wo_t": np.ascontiguousarray(Wo[:, sl].T.astype(bf16)),
        }
        if has_bias:
            m["bq_t"] = np.ascontiguousarray(bq[sl] * SCALING)
            m["bk_t"] = np.ascontiguousarray(bk[sl])
            m["bv_t"] = np.ascontiguousarray(bv[sl])
        if has_mask:
            mb = np.broadcast_to(mask[b].reshape(-1, mask.shape[-2], mask.shape[-1])[0],
                                 (q.shape[1], kv.shape[1]))
            m["mask_t"] = np.ascontiguousarray(mb.T)
        in_maps.append(m)

    if PROFILE:
        _install_profile_hook()
    res = run_bass_kernel_spmd(nc, in_maps, core_ids=list(range(N_CORES)),
                               trace=bool(PROFILE))
    LAST_EXEC_NS = res.exec_time_ns
    LAST_RESULTS = res
    outs = [res.results[c]["y"].astype(np.float32) for c in range(N_CORES)]
    y = np.stack([sum(outs[b * TPG:(b + 1) * TPG]) for b in range(DP)])
    return (y + bo).astype(np.float32)


# revision 38
# speedup vs baseline: 1.0383x; 1.0323x over previous
"""Multi-head attention (B=2, S=T=2048, D=1024, H=16) on 8 TRN2 NeuronCores.

Sharding: 2-way data parallel over batch x 4-way tensor parallel over heads.
Core c handles batch c//4 and heads [4*(c%4), 4*(c%4)+4).

All matmuls run in bf16 with fp32 PSUM. fp8 was evaluated and rejected:
relative noise on attention weights or on v passes ~1:1 into the output
(the output is itself a weighted average, so averaging shrinks signal and
noise together) — e5m2 exp'd scores alone measure 6.2e-2 L2, over the
2e-2 gate. DoubleRow fp8 also only reaches ~263ns per 512-col instr on
hardware (vs 259ns bf16): matmul streaming is moving-operand-bandwidth
bound, so fp8 buys no tensor time either.

Each head's 64 v columns sit next to a 64-wide all-ones block, so the
PV matmul also emits the softmax denominator replicated across 64
partitions; normalization is a copy + reciprocal + multiply on the DVE.
The two QKT matmuls of a head pair use 64-partition stationaries in
disjoint PE row halves and execute concurrently (~387ns for both).

Schedule: only kT[m0] + qT[m0, first t-block] projections run before the
attention stream starts (first exp ~25us vs ~59us when all projections
run upfront). The remaining q/k projections, the v projection, and the
deferred out-projection chunks drip into the attention loop's spare
tensor slots. All PSUM drains run on the DVE so the ACT engine is
exp-only during the stream.
"""

import sys
import types

import numpy as np

import concourse.bass as bass  # noqa: F401  (registers engine classes)
import concourse.tile as tile
import concourse.mybir as mybir
from concourse import bacc
from concourse.bass import ts
from concourse.bass_utils import run_bass_kernel_spmd

FP32 = mybir.dt.float32
BF16 = mybir.dt.bfloat16
AF = mybir.ActivationFunctionType

D_MODEL = 1024
NUM_HEADS = 16
D_HEAD = 64
SCALING = D_HEAD ** -0.5
N_CORES = 8
DP = 2                      # data-parallel over batch
TPG = N_CORES // DP         # 4 tensor-parallel groups
DC = D_MODEL // TPG         # 256 output dims per core
HPC = DC // D_HEAD          # 4 heads per core

PV_PLAG = 2                 # PV lag in sc-pairs behind the exp stream

PROFILE = False             # set by test harness; collects exec_time_ns
LAST_EXEC_NS = None
LAST_RESULTS = None

_programs = {}


def _install_profile_hook():
    if "antenv.axon_hooks" in sys.modules:
        return
    try:
        from trn_agent_boot.trn_boot import _ntff_profile_via_ctypes
        hook = _ntff_profile_via_ctypes("/opt/axon/libaxon_pjrt.so")
    except Exception:
        hook = None
    mod = types.ModuleType("antenv.axon_hooks")
    mod.get_axon_ntff_profile_hook = lambda: hook
    mod.set_axon_ntff_profile_hook = lambda h: None
    sys.modules["antenv.axon_hooks"] = mod


def build_program(has_bias=False, has_mask=False, T=2048, S=2048, D=D_MODEL,
                  DCL=DC, TT=512):
    """Build the per-core bass program (SPMD: same program, per-core inputs)."""
    KC = D // 128            # contraction chunks
    SC = S // 128            # s chunks (PV contraction)
    NTT = T // TT            # t tiles
    MC = DCL // 128          # qT/kT partition chunks
    HP = (DCL // D_HEAD) // 2  # head pairs
    PT = 2 * TT              # projection tile width (psS-shaped)
    NET = D // TT            # out-proj e tiles

    nc = bacc.Bacc("TRN2", target_bir_lowering=False, debug=False)
    xq_t = nc.dram_tensor("xq_t", [D, T], BF16, kind="ExternalInput")
    xkv_t = nc.dram_tensor("xkv_t", [D, S], BF16, kind="ExternalInput")
    wq_t = nc.dram_tensor("wq_t", [D, DCL], BF16, kind="ExternalInput")
    wk_t = nc.dram_tensor("wk_t", [D, DCL], BF16, kind="ExternalInput")
    wv_t = nc.dram_tensor("wv_t", [D, DCL], BF16, kind="ExternalInput")
    wo_t = nc.dram_tensor("wo_t", [DCL, D], BF16, kind="ExternalInput")
    y_t = nc.dram_tensor("y", [T, D], BF16, kind="ExternalOutput")
    if has_bias:
        bq_t = nc.dram_tensor("bq_t", [DCL], FP32, kind="ExternalInput")
        bk_t = nc.dram_tensor("bk_t", [DCL], FP32, kind="ExternalInput")
        bv_t = nc.dram_tensor("bv_t", [DCL], FP32, kind="ExternalInput")
    if has_mask:
        mask_t = nc.dram_tensor("mask_t", [S, T], FP32, kind="ExternalInput")

    with tile.TileContext(nc) as tc:
        with tc.tile_pool(name="w", bufs=1) as wpool, \
             tc.tile_pool(name="big", bufs=1) as big, \
             tc.tile_pool(name="r", bufs=4) as rpool, \
             tc.tile_pool(name="yst", bufs=3) as ypool, \
             tc.tile_pool(name="psS", bufs=2, space="PSUM") as psS, \
             tc.tile_pool(name="psC", bufs=2, space="PSUM") as psC, \
             tc.tile_pool(name="psO", bufs=2, space="PSUM") as psO, \
             (tc.tile_pool(name="msk", bufs=4) if has_mask else _nullpool()) as mpool:

            # ---- persistent weights / activations ----
            wk_sb = wpool.tile([128, KC, DCL], BF16, tag="wk")
            wq_sb = wpool.tile([128, KC, DCL], BF16, tag="wq")
            wv_sb = wpool.tile([128, KC, DCL], BF16, tag="wv")
            wo_sb = wpool.tile([128, MC, D], BF16, tag="wo")
            xkv_sb = big.tile([128, KC, S], BF16, tag="xkv")
            xq_sb = big.tile([128, KC, T], BF16, tag="xq")
            kT_sb = big.tile([128, MC, S], BF16, tag="kT")
            qT_sb = big.tile([128, MC, T], BF16, tag="qT")
            ct_sb = big.tile([128, MC, T], BF16, tag="ct")
            # v blocks: [head0 d64 | ones64 | head1 d64 | ones64 | ...]
            v_sb = big.tile([128, SC, 2 * HPC, D_HEAD], BF16, tag="v")
            # exp'd scores ring, one slot per key chunk of the live group
            e_ring = big.tile([128, SC, 2 * TT], BF16, tag="er")

            ones_col_f = wpool.tile([128, 1], FP32, tag="onescolf")
            nc.gpsimd.memset(ones_col_f[:], 1.0)
            ones_col = wpool.tile([128, 1], BF16, tag="onescol")
            nc.vector.tensor_copy(ones_col[:], ones_col_f[:])

            # ---- PE warm-up: the tensor engine reaches full clock only
            # after ~3us of continuous execution. Dummy matmuls during the
            # input-DMA window ramp it so the real projections run at speed.
            warm = wpool.tile([128, 512], BF16, tag="warm")
            nc.gpsimd.memset(warm[:], 0.25)
            for _ in range(9):
                pw = psS.tile([128, PT], FP32, tag="s")
                nc.tensor.matmul(pw[:, 0:TT], warm[:, 0:128], warm[:],
                                 start=True, stop=True)
                nc.tensor.matmul(pw[:, TT:2 * TT], warm[:, 0:128], warm[:],
                                 start=True, stop=True)
            # v viewed as [p, sc, head, (data|ones), c]
            v5 = v_sb[:].rearrange("p s (h two) c -> p s h two c", two=2)
            nc.vector.tensor_copy(
                v5[:, :, :, 1, :],
                ones_col[:].to_broadcast((128, SC, HPC, 1, D_HEAD)),
            )

            # ---- input DMAs ----
            # Queues run at ~30-35 GB/s each, so a 512KB weight on one queue
            # gates the first projection by ~15us. The first wave spreads the
            # critical bytes (wk pieces, the first xkv column-half, wq
            # pieces) across all 16 queues; later waves follow first use.
            def dma_w(w_sb, w_t, pieces=1):
                for p in range(pieces):
                    kc = KC // pieces
                    nc.sync.dma_start(
                        w_sb[:, p * kc:(p + 1) * kc, :],
                        w_t.ap().rearrange("(c p) d -> p c d", p=128)
                        [:, p * kc:(p + 1) * kc, :])

            dma_w(wk_sb, wk_t, pieces=4)
            for c in range(KC):
                nc.sync.dma_start(
                    xkv_sb[:, c, 0:S // 2],
                    xkv_t.ap()[c * 128:(c + 1) * 128, 0:S // 2])
            dma_w(wq_sb, wq_t, pieces=4)
            # second wave: the upfront qT block's xq columns, then wv for
            # the dripped v-projection, then the rest
            for c in range(KC):
                nc.sync.dma_start(
                    xq_sb[:, c, 0:TT], xq_t.ap()[c * 128:(c + 1) * 128, 0:TT])
            dma_w(wv_sb, wv_t, pieces=4)
            for c in range(KC):
                nc.sync.dma_start(
                    xkv_sb[:, c, S // 2:S],
                    xkv_t.ap()[c * 128:(c + 1) * 128, S // 2:S])
            for c in range(KC):
                nc.sync.dma_start(
                    xq_sb[:, c, TT:T], xq_t.ap()[c * 128:(c + 1) * 128, TT:T])
            if has_bias:
                bq_sb = wpool.tile([128, MC], FP32, tag="bq")
                nc.sync.dma_start(bq_sb[:], bq_t.ap().rearrange("(m p) -> p m", p=128))
                bk_sb = wpool.tile([128, MC], FP32, tag="bk")
                nc.sync.dma_start(bk_sb[:], bk_t.ap().rearrange("(m p) -> p m", p=128))
                bv_sb = wpool.tile([128, MC], FP32, tag="bv")
                nc.sync.dma_start(bv_sb[:], bv_t.ap().rearrange("(m p) -> p m", p=128))
            nc.sync.dma_start(
                wo_sb[:], wo_t.ap().rearrange("(m p) e -> p m e", p=128))

            # ---- kT/qT projections (1024-wide PSUM tiles, DVE drains) ----
            def proj(dst_m, w_sb_, x_sb, m, hh, pool, bias=None):
                # one 512-wide block (the matmul ISA caps the moving dim at
                # 512). Upfront blocks go through psS; blocks dripped into
                # the attention loop use psO so they never block the score
                # tiles' 2-slot psS rotation (which would stall the exps).
                ps = pool.tile([128, TT], FP32,
                               tag="s" if pool is psS else "o")
                for c in range(KC):
                    nc.tensor.matmul(
                        ps[:], w_sb_[:, c, ts(m, 128)],
                        x_sb[:, c, ts(hh, TT)],
                        start=(c == 0), stop=(c == KC - 1))
                dst = dst_m[:, ts(hh, TT)]
                if bias is not None:
                    nc.vector.tensor_scalar_add(dst, ps[:], bias)
                else:
                    nc.vector.tensor_copy(dst, ps[:])

            def emit_proj(which, m, hh, pool=psS):
                if which == 0:
                    proj(kT_sb[:, m, :], wk_sb, xkv_sb, m, hh, pool,
                         bk_sb[:, m:m + 1] if has_bias else None)
                else:
                    proj(qT_sb[:, m, :], wq_sb, xq_sb, m, hh, pool,
                         bq_sb[:, m:m + 1] if has_bias else None)

            # upfront (dense, full-clock after warmup): just the first
            # s-block of kT[m0] and t-block of qT[m0]. kT's later s-blocks
            # (first used at sc 4/8/12) drip through the then-idle psO pool
            # in the first iterations; everything else drips in first-use
            # order (kT/qT m1 at group (0,1) = iter 16+, qT later t-blocks
            # at t-tiles 1-3).
            emit_proj(0, 0, 0)
            emit_proj(0, 0, 1)
            emit_proj(1, 0, 0)
            early_proj = [(0, 0, 2), (0, 0, 3)]
            pending_proj = [(0, 1, 0), (0, 1, 1), (1, 1, 0), (0, 1, 2),
                            (0, 1, 3), (1, 0, 1), (1, 1, 1), (1, 0, 2),
                            (1, 1, 2), (1, 0, 3), (1, 1, 3)]

            # ---- v projection chunk (dripped into the first group) ----
            def emit_vproj(sc):
                ps = psO.tile([128, TT], FP32, tag="o")
                psv = ps[:, 0:DCL]
                last = KC - 1
                for c in range(KC):
                    nc.tensor.matmul(
                        psv, xkv_sb[:, c, ts(sc, 128)], wv_sb[:, c, :],
                        start=(c == 0), stop=(c == last and not has_bias))
                pshc = psv.rearrange("p (h c) -> p h c", c=D_HEAD)
                if has_bias:
                    # bias via DVE add during the copy
                    nc.vector.tensor_scalar_add(
                        v5[:, sc, :, 0, :], pshc, bv_sb[:, 0:1])
                else:
                    nc.vector.tensor_copy(v5[:, sc, :, 0, :], pshc)

            # ---- out-projection chunks (deferred, dripped) ----
            def emit_outproj_chunk(tq, et, on_act=False, pool=None):
                pool = psO if pool is None else pool
                ysb = ypool.tile([128, TT], BF16, tag="y", name=f"ysb{tq}_{et}")
                ps = pool.tile([128, TT], FP32,
                               tag="s" if pool is psS else "o")
                for m in range(MC):
                    nc.tensor.matmul(
                        ps[:], ct_sb[:, m, ts(tq, 128)], wo_sb[:, m, ts(et, TT)],
                        start=(m == 0), stop=(m == MC - 1))
                if on_act:
                    nc.scalar.activation(ysb[:], ps[:], AF.Copy)
                else:
                    nc.vector.tensor_copy(ysb[:], ps[:])
                nc.sync.dma_start(y_t.ap()[ts(tq, 128), ts(et, TT)], ysb[:])

            # ---- attention: QKT/exp stream + lagged PV queue ----
            ctx = {}            # (tt, hp) -> [cA, cB]

            def emit_den(cps, lh, tt, hp):
                # copy numerators out too so the PSUM slot frees after two
                # short copies free the PSUM slot for the next group's PV
                den = rpool.tile([64, TT], FP32, tag="den")
                nc.vector.tensor_copy(den[:], cps[64:128, :])
                num = rpool.tile([64, TT], FP32, tag="num")
                nc.vector.tensor_copy(num[:], cps[0:64, :])
                rec = rpool.tile([64, TT], FP32, tag="rec")
                nc.vector.reciprocal_approx_fast(rec[:], den[:])
                dst = ct_sb[lh:lh + 64, hp, ts(tt, TT)]
                nc.vector.tensor_mul(dst, num[:], rec[:])

            def pop_pv(pvq):
                tt, hp, sc = pvq.pop(0)
                ha, hb = 2 * hp, 2 * hp + 1
                if sc == 0:
                    ctx[(tt, hp)] = [
                        psC.tile([128, TT], FP32, tag="c", name=f"cA{tt}_{hp}"),
                        psC.tile([128, TT], FP32, tag="c", name=f"cB{tt}_{hp}"),
                    ]
                cA, cB = ctx[(tt, hp)]
                start, stop = (sc == 0), (sc == SC - 1)
                nc.tensor.matmul(
                    cA[:], v_sb[:, sc, 2 * ha:2 * ha + 2, :],
                    e_ring[:, sc, 0:TT], start=start, stop=stop)
                nc.tensor.matmul(
                    cB[:], v_sb[:, sc, 2 * hb:2 * hb + 2, :],
                    e_ring[:, sc, TT:2 * TT], start=start, stop=stop)
                if stop:
                    emit_den(cA, 0, tt, hp)
                    emit_den(cB, 64, tt, hp)
                    del ctx[(tt, hp)]

            pvq = []
            deferred = []
            for tt in range(NTT):
                for hp in range(HP):
                    for sc in range(SC):
                        it = hp * SC + sc
                        sAB = psS.tile([128, 2 * TT], FP32, tag="s")
                        nc.tensor.matmul(
                            sAB[:, 0:TT], kT_sb[0:64, hp, ts(sc, 128)],
                            qT_sb[0:64, hp, ts(tt, TT)], start=True, stop=True)
                        nc.tensor.matmul(
                            sAB[:, TT:2 * TT], kT_sb[64:128, hp, ts(sc, 128)],
                            qT_sb[64:128, hp, ts(tt, TT)], start=True, stop=True)
                        if has_mask:
                            mt = mpool.tile([128, TT], FP32, tag="m")
                            nc.sync.dma_start(
                                mt[:], mask_t.ap()[ts(sc, 128), ts(tt, TT)])
                            nc.vector.tensor_add(sAB[:, 0:TT], sAB[:, 0:TT], mt[:])
                            nc.vector.tensor_add(sAB[:, TT:2 * TT],
                                                 sAB[:, TT:2 * TT], mt[:])
                        nc.scalar.activation(e_ring[:, sc, :], sAB[:], AF.Exp)
                        pvq.append((tt, hp, sc))
                        # fillers behind this iteration's QKT: v-projection
                        # chunks (shifted 3 late so the first exps aren't
                        # tensor-bound), the remaining kT/qT blocks, the
                        # deferred out-projection chunks.
                        if tt == 0 and it < 3 and early_proj:
                            emit_proj(*early_proj.pop(0), pool=psO)
                        if tt == 0 and 3 <= it < 3 + SC:
                            emit_vproj(it - 3)
                        if tt == 0 and sc % 2 == 0 and pending_proj:
                            emit_proj(*pending_proj.pop(0))
                        # ct of t-tile tt-1 is final only a few iterations
                        # into (tt, hp0) — keep hp0 drains late, hp1 early.
                        drain = (sc % 2 == 1) and \
                            (sc >= 7 if hp == 0 else sc <= 7)
                        if deferred and drain:
                            emit_outproj_chunk(*deferred.pop(0))
                        if len(pvq) > 2 * PV_PLAG:
                            pop_pv(pvq)
                # queue this t-tile's output projection
                for tq in range(tt * (TT // 128), (tt + 1) * (TT // 128)):
                    for et in range(NET):
                        deferred.append((tq, et))
            while pvq:
                pop_pv(pvq)
            # tail out-projections: psS is free once the exps are done, so
            # alternate psO/psS for a 4-deep chunk pipeline
            for i, (tq, et) in enumerate(deferred):
                emit_outproj_chunk(tq, et, on_act=(i % 2 == 0),
                                   pool=(psS if i % 2 else psO))

    nc.compile()
    return nc


class _nullpool:
    def __enter__(self):
        return None

    def __exit__(self, *a):
        return False


def _get_program(has_bias, has_mask):
    key = (has_bias, has_mask)
    if key not in _programs:
        _programs[key] = build_program(has_bias, has_mask)
    return _programs[key]


def kernel(query_states, key_value_states, attention_mask,
           Wq, bq, Wk, bk, Wv, bv, Wo, bo):
    global LAST_EXEC_NS, LAST_RESULTS
    import ml_dtypes
    bf16 = ml_dtypes.bfloat16
    q = np.asarray(query_states, dtype=np.float32)
    kv = np.asarray(key_value_states, dtype=np.float32)
    mask = np.asarray(attention_mask, dtype=np.float32)
    Wq = np.asarray(Wq, np.float32); bq = np.asarray(bq, np.float32)
    Wk = np.asarray(Wk, np.float32); bk = np.asarray(bk, np.float32)
    Wv = np.asarray(Wv, np.float32); bv = np.asarray(bv, np.float32)
    Wo = np.asarray(Wo, np.float32); bo = np.asarray(bo, np.float32)

    has_bias = bool(np.any(bq) or np.any(bk) or np.any(bv))
    has_mask = bool(np.any(mask))
    nc = _get_program(has_bias, has_mask)

    # per-batch activations (shared across the 4 TP cores of each batch)
    xq16 = [np.ascontiguousarray(q[b].T.astype(bf16)) for b in range(DP)]
    xkv16 = [np.ascontiguousarray(kv[b].T.astype(bf16)) for b in range(DP)]

    in_maps = []
    for c in range(N_CORES):
        b, hg = divmod(c, TPG)
        sl = slice(DC * hg, DC * (hg + 1))
        m = {
            "xq_t": xq16[b],
            "xkv_t": xkv16[b],
            "wq_t": np.ascontiguousarray((Wq[sl] * SCALING).T.astype(bf16)),
            "wk_t": np.ascontiguousarray(Wk[sl].T.astype(bf16)),
            "wv_t": np.ascontiguousarray(Wv[sl].T.astype(bf16)),
            "wo_t": np.ascontiguousarray(Wo[:, sl].T.astype(bf16)),
        }
        if has_bias:
            m["bq_t"] = np.ascontiguousarray(bq[sl] * SCALING)
            m["bk_t"] = np.ascontiguousarray(bk[sl])
            m["bv_t"] = np.ascontiguousarray(bv[sl])
        if has_mask:
            mb = np.broadcast_to(mask[b].reshape(-1, mask.shape[-2], mask.shape[-1])[0],
                                 (q.shape[1], kv.shape[1]))
            m["mask_t"] = np.ascontiguousarray(mb.T)
        in_maps.append(m)

    if PROFILE:
        _install_profile_hook()
    res = run_bass_kernel_spmd(nc, in_maps, core_ids=list(range(N_CORES)),
                               trace=bool(PROFILE))
    LAST_EXEC_NS = res.exec_time_ns
    LAST_RESULTS = res
    outs = [res.results[c]["y"].astype(np.float32) for c in range(N_CORES)]
    y = np.stack([sum(outs[b * TPG:(b + 1) * TPG]) for b in range(DP)])
    return (y + bo).astype(np.float32)


# revision 39
# speedup vs baseline: 1.0515x; 1.0127x over previous
"""Multi-head attention (B=2, S=T=2048, D=1024, H=16) on 8 TRN2 NeuronCores.

Sharding: 2-way data parallel over batch x 4-way tensor parallel over heads.
Core c handles batch c//4 and heads [4*(c%4), 4*(c%4)+4).

All matmuls run in bf16 with fp32 PSUM. fp8 was evaluated and rejected:
relative noise on attention weights or on v passes ~1:1 into the output
(the output is itself a weighted average, so averaging shrinks signal and
noise together) — e5m2 exp'd scores alone measure 6.2e-2 L2, over the
2e-2 gate. DoubleRow fp8 also only reaches ~263ns per 512-col instr on
hardware (vs 259ns bf16): matmul streaming is moving-operand-bandwidth
bound, so fp8 buys no tensor time either.

Each head's 64 v columns sit next to a 64-wide all-ones block, so the
PV matmul also emits the softmax denominator replicated across 64
partitions; normalization is a copy + reciprocal + multiply on the DVE.
The two QKT matmuls of a head pair use 64-partition stationaries in
disjoint PE row halves and execute concurrently (~387ns for both).

Schedule: only kT[m0] + qT[m0, first t-block] projections run before the
attention stream starts (first exp ~25us vs ~59us when all projections
run upfront). The remaining q/k projections, the v projection, and the
deferred out-projection chunks drip into the attention loop's spare
tensor slots. All PSUM drains run on the DVE so the ACT engine is
exp-only during the stream.
"""

import sys
import types

import numpy as np

import concourse.bass as bass  # noqa: F401  (registers engine classes)
import concourse.tile as tile
import concourse.mybir as mybir
from concourse import bacc
from concourse.bass import ts
from concourse.bass_utils import run_bass_kernel_spmd

FP32 = mybir.dt.float32
BF16 = mybir.dt.bfloat16
AF = mybir.ActivationFunctionType

D_MODEL = 1024
NUM_HEADS = 16
D_HEAD = 64
SCALING = D_HEAD ** -0.5
N_CORES = 8
DP = 2                      # data-parallel over batch
TPG = N_CORES // DP         # 4 tensor-parallel groups
DC = D_MODEL // TPG         # 256 output dims per core
HPC = DC // D_HEAD          # 4 heads per core

PV_PLAG = 2                 # PV lag in sc-pairs behind the exp stream

PROFILE = False             # set by test harness; collects exec_time_ns
LAST_EXEC_NS = None
LAST_RESULTS = None

_programs = {}


def _install_profile_hook():
    if "antenv.axon_hooks" in sys.modules:
        return
    try:
        from trn_agent_boot.trn_boot import _ntff_profile_via_ctypes
        hook = _ntff_profile_via_ctypes("/opt/axon/libaxon_pjrt.so")
    except Exception:
        hook = None
    mod = types.ModuleType("antenv.axon_hooks")
    mod.get_axon_ntff_profile_hook = lambda: hook
    mod.set_axon_ntff_profile_hook = lambda h: None
    sys.modules["antenv.axon_hooks"] = mod


def build_program(has_bias=False, has_mask=False, T=2048, S=2048, D=D_MODEL,
                  DCL=DC, TT=512):
    """Build the per-core bass program (SPMD: same program, per-core inputs)."""
    KC = D // 128            # contraction chunks
    SC = S // 128            # s chunks (PV contraction)
    NTT = T // TT            # t tiles
    MC = DCL // 128          # qT/kT partition chunks
    HP = (DCL // D_HEAD) // 2  # head pairs
    PT = 2 * TT              # projection tile width (psS-shaped)
    NET = D // TT            # out-proj e tiles

    nc = bacc.Bacc("TRN2", target_bir_lowering=False, debug=False)
    xq_t = nc.dram_tensor("xq_t", [D, T], BF16, kind="ExternalInput")
    xkv_t = nc.dram_tensor("xkv_t", [D, S], BF16, kind="ExternalInput")
    wq_t = nc.dram_tensor("wq_t", [D, DCL], BF16, kind="ExternalInput")
    wk_t = nc.dram_tensor("wk_t", [D, DCL], BF16, kind="ExternalInput")
    wv_t = nc.dram_tensor("wv_t", [D, DCL], BF16, kind="ExternalInput")
    wo_t = nc.dram_tensor("wo_t", [DCL, D], BF16, kind="ExternalInput")
    y_t = nc.dram_tensor("y", [T, D], BF16, kind="ExternalOutput")
    if has_bias:
        bq_t = nc.dram_tensor("bq_t", [DCL], FP32, kind="ExternalInput")
        bk_t = nc.dram_tensor("bk_t", [DCL], FP32, kind="ExternalInput")
        bv_t = nc.dram_tensor("bv_t", [DCL], FP32, kind="ExternalInput")
    if has_mask:
        mask_t = nc.dram_tensor("mask_t", [S, T], FP32, kind="ExternalInput")

    with tile.TileContext(nc) as tc:
        with tc.tile_pool(name="w", bufs=1) as wpool, \
             tc.tile_pool(name="big", bufs=1) as big, \
             tc.tile_pool(name="r", bufs=4) as rpool, \
             tc.tile_pool(name="yst", bufs=3) as ypool, \
             tc.tile_pool(name="psS", bufs=2, space="PSUM") as psS, \
             tc.tile_pool(name="psC", bufs=2, space="PSUM") as psC, \
             tc.tile_pool(name="psO", bufs=2, space="PSUM") as psO, \
             (tc.tile_pool(name="msk", bufs=4) if has_mask else _nullpool()) as mpool:

            # ---- persistent weights / activations ----
            wk_sb = wpool.tile([128, KC, DCL], BF16, tag="wk")
            wq_sb = wpool.tile([128, KC, DCL], BF16, tag="wq")
            wv_sb = wpool.tile([128, KC, DCL], BF16, tag="wv")
            wo_sb = wpool.tile([128, MC, D], BF16, tag="wo")
            xkv_sb = big.tile([128, KC, S], BF16, tag="xkv")
            xq_sb = big.tile([128, KC, T], BF16, tag="xq")
            kT_sb = big.tile([128, MC, S], BF16, tag="kT")
            qT_sb = big.tile([128, MC, T], BF16, tag="qT")
            ct_sb = big.tile([128, MC, T], BF16, tag="ct")
            # v blocks: [head0 d64 | ones64 | head1 d64 | ones64 | ...]
            v_sb = big.tile([128, SC, 2 * HPC, D_HEAD], BF16, tag="v")
            # exp'd scores ring, one slot per key chunk of the live group
            e_ring = big.tile([128, SC, 2 * TT], BF16, tag="er")

            ones_col_f = wpool.tile([128, 1], FP32, tag="onescolf")
            nc.gpsimd.memset(ones_col_f[:], 1.0)
            ones_col = wpool.tile([128, 1], BF16, tag="onescol")
            nc.vector.tensor_copy(ones_col[:], ones_col_f[:])

            # ---- PE warm-up: the tensor engine reaches full clock only
            # after ~3us of continuous execution. Dummy matmuls during the
            # input-DMA window ramp it so the real projections run at speed.
            warm = wpool.tile([128, 512], BF16, tag="warm")
            nc.gpsimd.memset(warm[:], 0.25)
            for _ in range(9):
                pw = psS.tile([128, PT], FP32, tag="s")
                nc.tensor.matmul(pw[:, 0:TT], warm[:, 0:128], warm[:],
                                 start=True, stop=True)
                nc.tensor.matmul(pw[:, TT:2 * TT], warm[:, 0:128], warm[:],
                                 start=True, stop=True)
            # v viewed as [p, sc, head, (data|ones), c]
            v5 = v_sb[:].rearrange("p s (h two) c -> p s h two c", two=2)
            nc.vector.tensor_copy(
                v5[:, :, :, 1, :],
                ones_col[:].to_broadcast((128, SC, HPC, 1, D_HEAD)),
            )

            # ---- input DMAs ----
            # Queues run at ~30-35 GB/s each, so a 512KB weight on one queue
            # gates the first projection by ~15us. The first wave spreads the
            # critical bytes (wk pieces, the first xkv column-half, wq
            # pieces) across all 16 queues; later waves follow first use.
            def dma_w(w_sb, w_t, pieces=1):
                for p in range(pieces):
                    kc = KC // pieces
                    nc.sync.dma_start(
                        w_sb[:, p * kc:(p + 1) * kc, :],
                        w_t.ap().rearrange("(c p) d -> p c d", p=128)
                        [:, p * kc:(p + 1) * kc, :])

            dma_w(wk_sb, wk_t, pieces=4)
            for c in range(KC):
                nc.sync.dma_start(
                    xkv_sb[:, c, 0:S // 2],
                    xkv_t.ap()[c * 128:(c + 1) * 128, 0:S // 2])
            dma_w(wq_sb, wq_t, pieces=4)
            # second wave: the upfront qT block's xq columns, then wv for
            # the dripped v-projection, then the rest
            for c in range(KC):
                nc.sync.dma_start(
                    xq_sb[:, c, 0:TT], xq_t.ap()[c * 128:(c + 1) * 128, 0:TT])
            dma_w(wv_sb, wv_t, pieces=4)
            for c in range(KC):
                nc.sync.dma_start(
                    xkv_sb[:, c, S // 2:S],
                    xkv_t.ap()[c * 128:(c + 1) * 128, S // 2:S])
            for c in range(KC):
                nc.sync.dma_start(
                    xq_sb[:, c, TT:T], xq_t.ap()[c * 128:(c + 1) * 128, TT:T])
            if has_bias:
                bq_sb = wpool.tile([128, MC], FP32, tag="bq")
                nc.sync.dma_start(bq_sb[:], bq_t.ap().rearrange("(m p) -> p m", p=128))
                bk_sb = wpool.tile([128, MC], FP32, tag="bk")
                nc.sync.dma_start(bk_sb[:], bk_t.ap().rearrange("(m p) -> p m", p=128))
                bv_sb = wpool.tile([128, MC], FP32, tag="bv")
                nc.sync.dma_start(bv_sb[:], bv_t.ap().rearrange("(m p) -> p m", p=128))
            nc.sync.dma_start(
                wo_sb[:], wo_t.ap().rearrange("(m p) e -> p m e", p=128))

            # ---- kT/qT projections (1024-wide PSUM tiles, DVE drains) ----
            def proj(dst_m, w_sb_, x_sb, m, hh, pool, bias=None):
                # one 512-wide block (the matmul ISA caps the moving dim at
                # 512). Upfront blocks go through psS; blocks dripped into
                # the attention loop use psO so they never block the score
                # tiles' 2-slot psS rotation (which would stall the exps).
                ps = pool.tile([128, TT], FP32,
                               tag="s" if pool is psS else "o")
                for c in range(KC):
                    nc.tensor.matmul(
                        ps[:], w_sb_[:, c, ts(m, 128)],
                        x_sb[:, c, ts(hh, TT)],
                        start=(c == 0), stop=(c == KC - 1))
                dst = dst_m[:, ts(hh, TT)]
                if bias is not None:
                    nc.vector.tensor_scalar_add(dst, ps[:], bias)
                else:
                    nc.vector.tensor_copy(dst, ps[:])

            def emit_proj(which, m, hh, pool=psS):
                if which == 0:
                    proj(kT_sb[:, m, :], wk_sb, xkv_sb, m, hh, pool,
                         bk_sb[:, m:m + 1] if has_bias else None)
                else:
                    proj(qT_sb[:, m, :], wq_sb, xq_sb, m, hh, pool,
                         bq_sb[:, m:m + 1] if has_bias else None)

            # upfront (dense, full-clock after warmup): just the first
            # s-block of kT[m0] and t-block of qT[m0]. kT's later s-blocks
            # (first used at sc 4/8/12) drip through the then-idle psO pool
            # in the first iterations; everything else drips in first-use
            # order (kT/qT m1 at group (0,1) = iter 16+, qT later t-blocks
            # at t-tiles 1-3).
            emit_proj(0, 0, 0)
            emit_proj(0, 0, 1)
            emit_proj(1, 0, 0)
            early_proj = [(0, 0, 2), (0, 0, 3)]
            pending_proj = [(0, 1, 0), (0, 1, 1), (1, 1, 0), (0, 1, 2),
                            (0, 1, 3), (1, 0, 1), (1, 1, 1), (1, 0, 2),
                            (1, 1, 2), (1, 0, 3), (1, 1, 3)]

            # ---- v projection chunk (dripped into the first group) ----
            def emit_vproj(sc):
                ps = psO.tile([128, TT], FP32, tag="o")
                psv = ps[:, 0:DCL]
                last = KC - 1
                for c in range(KC):
                    nc.tensor.matmul(
                        psv, xkv_sb[:, c, ts(sc, 128)], wv_sb[:, c, :],
                        start=(c == 0), stop=(c == last and not has_bias))
                pshc = psv.rearrange("p (h c) -> p h c", c=D_HEAD)
                if has_bias:
                    # bias via DVE add during the copy
                    nc.vector.tensor_scalar_add(
                        v5[:, sc, :, 0, :], pshc, bv_sb[:, 0:1])
                else:
                    nc.vector.tensor_copy(v5[:, sc, :, 0, :], pshc)

            # ---- out-projection chunks (deferred, dripped) ----
            def emit_outproj_chunk(tq, et, on_act=False, pool=None):
                pool = psO if pool is None else pool
                ysb = ypool.tile([128, TT], BF16, tag="y", name=f"ysb{tq}_{et}")
                ps = pool.tile([128, TT], FP32,
                               tag="s" if pool is psS else "o")
                for m in range(MC):
                    nc.tensor.matmul(
                        ps[:], ct_sb[:, m, ts(tq, 128)], wo_sb[:, m, ts(et, TT)],
                        start=(m == 0), stop=(m == MC - 1))
                if on_act:
                    nc.scalar.activation(ysb[:], ps[:], AF.Copy)
                else:
                    nc.vector.tensor_copy(ysb[:], ps[:])
                nc.sync.dma_start(y_t.ap()[ts(tq, 128), ts(et, TT)], ysb[:])

            # ---- attention: QKT/exp stream + lagged PV queue ----
            ctx = {}            # (tt, hp) -> [cA, cB]

            def emit_den(cps, lh, tt, hp):
                # copy numerators out too so the PSUM slot frees after two
                # short copies free the PSUM slot for the next group's PV
                den = rpool.tile([64, TT], FP32, tag="den")
                nc.vector.tensor_copy(den[:], cps[64:128, :])
                num = rpool.tile([64, TT], FP32, tag="num")
                nc.vector.tensor_copy(num[:], cps[0:64, :])
                rec = rpool.tile([64, TT], FP32, tag="rec")
                nc.vector.reciprocal_approx_fast(rec[:], den[:])
                dst = ct_sb[lh:lh + 64, hp, ts(tt, TT)]
                nc.vector.tensor_mul(dst, num[:], rec[:])

            def pop_pv(pvq):
                tt, hp, sc = pvq.pop(0)
                ha, hb = 2 * hp, 2 * hp + 1
                if sc == 0:
                    ctx[(tt, hp)] = [
                        psC.tile([128, TT], FP32, tag="c", name=f"cA{tt}_{hp}"),
                        psC.tile([128, TT], FP32, tag="c", name=f"cB{tt}_{hp}"),
                    ]
                cA, cB = ctx[(tt, hp)]
                start, stop = (sc == 0), (sc == SC - 1)
                nc.tensor.matmul(
                    cA[:], v_sb[:, sc, 2 * ha:2 * ha + 2, :],
                    e_ring[:, sc, 0:TT], start=start, stop=stop)
                nc.tensor.matmul(
                    cB[:], v_sb[:, sc, 2 * hb:2 * hb + 2, :],
                    e_ring[:, sc, TT:2 * TT], start=start, stop=stop)
                if stop:
                    emit_den(cA, 0, tt, hp)
                    emit_den(cB, 64, tt, hp)
                    del ctx[(tt, hp)]

            pvq = []
            deferred = []
            for tt in range(NTT):
                for hp in range(HP):
                    for sc in range(SC):
                        it = hp * SC + sc
                        sAB = psS.tile([128, 2 * TT], FP32, tag="s")
                        nc.tensor.matmul(
                            sAB[:, 0:TT], kT_sb[0:64, hp, ts(sc, 128)],
                            qT_sb[0:64, hp, ts(tt, TT)], start=True, stop=True)
                        nc.tensor.matmul(
                            sAB[:, TT:2 * TT], kT_sb[64:128, hp, ts(sc, 128)],
                            qT_sb[64:128, hp, ts(tt, TT)], start=True, stop=True)
                        if has_mask:
                            mt = mpool.tile([128, TT], FP32, tag="m")
                            nc.sync.dma_start(
                                mt[:], mask_t.ap()[ts(sc, 128), ts(tt, TT)])
                            nc.vector.tensor_add(sAB[:, 0:TT], sAB[:, 0:TT], mt[:])
                            nc.vector.tensor_add(sAB[:, TT:2 * TT],
                                                 sAB[:, TT:2 * TT], mt[:])
                        nc.scalar.activation(e_ring[:, sc, :], sAB[:], AF.Exp)
                        pvq.append((tt, hp, sc))
                        # fillers behind this iteration's QKT: v-projection
                        # chunks (shifted 3 late so the first exps aren't
                        # tensor-bound), the remaining kT/qT blocks, the
                        # deferred out-projection chunks.
                        if tt == 0 and it < 3 and early_proj:
                            emit_proj(*early_proj.pop(0), pool=psO)
                        if tt == 0 and 3 <= it < 3 + SC:
                            emit_vproj(it - 3)
                        if tt == 0 and sc % 2 == 0 and pending_proj:
                            emit_proj(*pending_proj.pop(0))
                        # PV pop first: its den chain gates the psC slot and
                        # ct, so it shouldn't queue behind out-proj drains
                        if len(pvq) > 2 * PV_PLAG:
                            pop_pv(pvq)
                        # ct of t-tile tt-1 is final only a few iterations
                        # into (tt, hp0) — keep hp0 drains late, hp1 early.
                        drain = (sc % 2 == 1) and \
                            (sc >= 7 if hp == 0 else sc <= 7)
                        if deferred and drain:
                            emit_outproj_chunk(*deferred.pop(0))
                # queue this t-tile's output projection
                for tq in range(tt * (TT // 128), (tt + 1) * (TT // 128)):
                    for et in range(NET):
                        deferred.append((tq, et))
            while pvq:
                pop_pv(pvq)
            # tail out-projections: psS is free once the exps are done, so
            # alternate psO/psS for a 4-deep chunk pipeline
            for i, (tq, et) in enumerate(deferred):
                emit_outproj_chunk(tq, et, on_act=(i % 2 == 0),
                                   pool=(psS if i % 2 else psO))

    nc.compile()
    return nc


class _nullpool:
    def __enter__(self):
        return None

    def __exit__(self, *a):
        return False


def _get_program(has_bias, has_mask):
    key = (has_bias, has_mask)
    if key not in _programs:
        _programs[key] = build_program(has_bias, has_mask)
    return _programs[key]


def kernel(query_states, key_value_states, attention_mask,
           Wq, bq, Wk, bk, Wv, bv, Wo, bo):
    global LAST_EXEC_NS, LAST_RESULTS
    import ml_dtypes
    bf16 = ml_dtypes.bfloat16
    q = np.asarray(query_states, dtype=np.float32)
    kv = np.asarray(key_value_states, dtype=np.float32)
    mask = np.asarray(attention_mask, dtype=np.float32)
    Wq = np.asarray(Wq, np.float32); bq = np.asarray(bq, np.float32)
    Wk = np.asarray(Wk, np.float32); bk = np.asarray(bk, np.float32)
    Wv = np.asarray(Wv, np.float32); bv = np.asarray(bv, np.float32)
    Wo = np.asarray(Wo, np.float32); bo = np.asarray(bo, np.float32)

    has_bias = bool(np.any(bq) or np.any(bk) or np.any(bv))
    has_mask = bool(np.any(mask))
    nc = _get_program(has_bias, has_mask)

    # per-batch activations (shared across the 4 TP cores of each batch)
    xq16 = [np.ascontiguousarray(q[b].T.astype(bf16)) for b in range(DP)]
    xkv16 = [np.ascontiguousarray(kv[b].T.astype(bf16)) for b in range(DP)]

    in_maps = []
    for c in range(N_CORES):
        b, hg = divmod(c, TPG)
        sl = slice(DC * hg, DC * (hg + 1))
        m = {
            "xq_t": xq16[b],
            "xkv_t": xkv16[b],
            "wq_t": np.ascontiguousarray((Wq[sl] * SCALING).T.astype(bf16)),
            "wk_t": np.ascontiguousarray(Wk[sl].T.astype(bf16)),
            "wv_t": np.ascontiguousarray(Wv[sl].T.astype(bf16)),
            "wo_t": np.ascontiguousarray(Wo[:, sl].T.astype(bf16)),
        }
        if has_bias:
            m["bq_t"] = np.ascontiguousarray(bq[sl] * SCALING)
            m["bk_t"] = np.ascontiguousarray(bk[sl])
            m["bv_t"] = np.ascontiguousarray(bv[sl])
        if has_mask:
            mb = np.broadcast_to(mask[b].reshape(-1, mask.shape[-2], mask.shape[-1])[0],
                                 (q.shape[1], kv.shape[1]))
            m["mask_t"] = np.ascontiguousarray(mb.T)
        in_maps.append(m)

    if PROFILE:
        _install_profile_hook()
    res = run_bass_kernel_spmd(nc, in_maps, core_ids=list(range(N_CORES)),
                               trace=bool(PROFILE))
    LAST_EXEC_NS = res.exec_time_ns
    LAST_RESULTS = res
    outs = [res.results[c]["y"].astype(np.float32) for c in range(N_CORES)]
    y = np.stack([sum(outs[b * TPG:(b + 1) * TPG]) for b in range(DP)])
    return (y + bo).astype(np.float32)


# revision 62
# speedup vs baseline: 1.0573x; 1.0055x over previous
"""Multi-head attention (B=2, S=T=2048, D=1024, H=16) on 8 TRN2 NeuronCores.

Sharding: 2-way data parallel over batch x 4-way tensor parallel over heads.
Core c handles batch c//4 and heads [4*(c%4), 4*(c%4)+4).

All matmuls run in bf16 with fp32 PSUM. fp8 was evaluated and rejected on
both axes: (1) numerics — relative noise on the attention weights or on v
passes ~1:1 into the output (the output is itself a weighted average, so
averaging shrinks signal and noise together); e5m2 exp'd scores alone
measure 6.2e-2 L2, over the 2e-2 gate, and fp8 q/k projections put 4.5e-2
noise on the scores. (2) speed — measured DoubleRow fp8 runs ~263ns per
512-col instr vs 259ns bf16: matmul streaming is moving-operand-bandwidth
bound, so fp8 buys almost no tensor time anyway.

Measured engine budget per core: tensor ~176us busy (the wall), ACT exp
~134us (128 x [128,1024] tiles at ~1048ns), DVE drains+den ~66us.
512-col matmuls run 216ns back-to-back at full clock; the PE p-state
ramps over ~3us of continuous work, so dummy warm-up matmuls run during
the input-DMA window. DMA queues move ~30-35GB/s each, so the critical
first-projection weights are split into 128KB pieces across queues.

Each head's 64 v columns sit next to a 64-wide all-ones block, so the
PV matmul also emits the softmax denominator replicated across 64
partitions (free: matmul cost is set by moving columns, not stationary
width); normalization is copy + reciprocal + multiply on the DVE, with
the PSUM-freeing copies first so the next group's PV reuses the slot.
The two QKT matmuls of a head pair use 64-partition stationaries in
disjoint PE row halves and execute concurrently (~320ns for both).

Schedule: only kT[m0 first half] + qT[m0 first t-block] project before
the attention stream starts (first exp ~19us); the remaining q/k
projections, the v projection, and the deferred out-projection chunks
drip into the attention loop (the Tile scheduler is dependency-driven,
so drips fill PE wait slots). PV consumption lags the exp stream via a
queue. PSUM is fully committed: scores 2x4KB, PV accumulators 2x2KB,
drip pool 2x2KB = 16KB; deeper buffering of any one pool starves the
others (measured worse). All stream-time PSUM drains run on the DVE so
the ACT engine is exp-only until the tail.
"""

import sys
import types

import numpy as np

import concourse.bass as bass  # noqa: F401  (registers engine classes)
import concourse.tile as tile
import concourse.mybir as mybir
from concourse import bacc
from concourse.bass import ts
from concourse.bass_utils import run_bass_kernel_spmd

FP32 = mybir.dt.float32
BF16 = mybir.dt.bfloat16
AF = mybir.ActivationFunctionType

D_MODEL = 1024
NUM_HEADS = 16
D_HEAD = 64
SCALING = D_HEAD ** -0.5
N_CORES = 8
DP = 2                      # data-parallel over batch
TPG = N_CORES // DP         # 4 tensor-parallel groups
DC = D_MODEL // TPG         # 256 output dims per core
HPC = DC // D_HEAD          # 4 heads per core

PV_PLAG = 2                 # PV lag in sc-pairs behind the exp stream

PROFILE = False             # set by test harness; collects exec_time_ns
LAST_EXEC_NS = None
LAST_RESULTS = None

_programs = {}


def _install_profile_hook():
    if "antenv.axon_hooks" in sys.modules:
        return
    try:
        from trn_agent_boot.trn_boot import _ntff_profile_via_ctypes
        hook = _ntff_profile_via_ctypes("/opt/axon/libaxon_pjrt.so")
    except Exception:
        hook = None
    mod = types.ModuleType("antenv.axon_hooks")
    mod.get_axon_ntff_profile_hook = lambda: hook
    mod.set_axon_ntff_profile_hook = lambda h: None
    sys.modules["antenv.axon_hooks"] = mod


def build_program(has_bias=False, has_mask=False, T=2048, S=2048, D=D_MODEL,
                  DCL=DC, TT=512):
    """Build the per-core bass program (SPMD: same program, per-core inputs)."""
    KC = D // 128            # contraction chunks
    SC = S // 128            # s chunks (PV contraction)
    NTT = T // TT            # t tiles
    MC = DCL // 128          # qT/kT partition chunks
    HP = (DCL // D_HEAD) // 2  # head pairs
    PT = 2 * TT              # projection tile width (psS-shaped)
    NET = D // TT            # out-proj e tiles

    nc = bacc.Bacc("TRN2", target_bir_lowering=False, debug=False)
    xq_t = nc.dram_tensor("xq_t", [D, T], BF16, kind="ExternalInput")
    xkv_t = nc.dram_tensor("xkv_t", [D, S], BF16, kind="ExternalInput")
    wq_t = nc.dram_tensor("wq_t", [D, DCL], BF16, kind="ExternalInput")
    wk_t = nc.dram_tensor("wk_t", [D, DCL], BF16, kind="ExternalInput")
    wv_t = nc.dram_tensor("wv_t", [D, DCL], BF16, kind="ExternalInput")
    wo_t = nc.dram_tensor("wo_t", [DCL, D], BF16, kind="ExternalInput")
    y_t = nc.dram_tensor("y", [T, D], BF16, kind="ExternalOutput")
    if has_bias:
        bq_t = nc.dram_tensor("bq_t", [DCL], FP32, kind="ExternalInput")
        bk_t = nc.dram_tensor("bk_t", [DCL], FP32, kind="ExternalInput")
        bv_t = nc.dram_tensor("bv_t", [DCL], FP32, kind="ExternalInput")
    if has_mask:
        mask_t = nc.dram_tensor("mask_t", [S, T], FP32, kind="ExternalInput")

    with tile.TileContext(nc) as tc:
        with tc.tile_pool(name="w", bufs=1) as wpool, \
             tc.tile_pool(name="big", bufs=1) as big, \
             tc.tile_pool(name="r", bufs=4) as rpool, \
             tc.tile_pool(name="yst", bufs=3) as ypool, \
             tc.tile_pool(name="psS", bufs=2, space="PSUM") as psS, \
             tc.tile_pool(name="psC", bufs=2, space="PSUM") as psC, \
             tc.tile_pool(name="psO", bufs=2, space="PSUM") as psO, \
             (tc.tile_pool(name="msk", bufs=4) if has_mask else _nullpool()) as mpool:

            # ---- persistent weights / activations ----
            wk_sb = wpool.tile([128, KC, DCL], BF16, tag="wk")
            wq_sb = wpool.tile([128, KC, DCL], BF16, tag="wq")
            wv_sb = wpool.tile([128, KC, DCL], BF16, tag="wv")
            wo_sb = wpool.tile([128, MC, D], BF16, tag="wo")
            xkv_sb = big.tile([128, KC, S], BF16, tag="xkv")
            xq_sb = big.tile([128, KC, T], BF16, tag="xq")
            kT_sb = big.tile([128, MC, S], BF16, tag="kT")
            qT_sb = big.tile([128, MC, T], BF16, tag="qT")
            ct_sb = big.tile([128, MC, T], BF16, tag="ct")
            # v blocks: [head0 d64 | ones64 | head1 d64 | ones64 | ...]
            v_sb = big.tile([128, SC, 2 * HPC, D_HEAD], BF16, tag="v")
            # exp'd scores ring, one slot per key chunk of the live group
            e_ring = big.tile([128, SC, 2 * TT], BF16, tag="er")

            ones_col_f = wpool.tile([128, 1], FP32, tag="onescolf")
            nc.gpsimd.memset(ones_col_f[:], 1.0)
            ones_col = wpool.tile([128, 1], BF16, tag="onescol")
            nc.vector.tensor_copy(ones_col[:], ones_col_f[:])

            # ---- PE warm-up: the tensor engine reaches full clock only
            # after ~3us of continuous execution. Dummy matmuls during the
            # input-DMA window ramp it so the real projections run at speed.
            warm = wpool.tile([128, 512], BF16, tag="warm")
            nc.gpsimd.memset(warm[:], 0.25)
            for _ in range(11):
                pw = psS.tile([128, PT], FP32, tag="s")
                nc.tensor.matmul(pw[:, 0:TT], warm[:, 0:128], warm[:],
                                 start=True, stop=True)
                nc.tensor.matmul(pw[:, TT:2 * TT], warm[:, 0:128], warm[:],
                                 start=True, stop=True)
            # v viewed as [p, sc, head, (data|ones), c]
            v5 = v_sb[:].rearrange("p s (h two) c -> p s h two c", two=2)
            nc.vector.tensor_copy(
                v5[:, :, :, 1, :],
                ones_col[:].to_broadcast((128, SC, HPC, 1, D_HEAD)),
            )

            # ---- input DMAs ----
            # Queues run at ~30-35 GB/s each, so a 512KB weight on one queue
            # gates the first projection by ~15us. The first wave spreads the
            # critical bytes (wk pieces, the first xkv column-half, wq
            # pieces) across all 16 queues; later waves follow first use.
            def dma_w(w_sb, w_t, pieces=1):
                for p in range(pieces):
                    kc = KC // pieces
                    nc.sync.dma_start(
                        w_sb[:, p * kc:(p + 1) * kc, :],
                        w_t.ap().rearrange("(c p) d -> p c d", p=128)
                        [:, p * kc:(p + 1) * kc, :])

            dma_w(wk_sb, wk_t, pieces=4)
            for c in range(KC):
                nc.sync.dma_start(
                    xkv_sb[:, c, 0:S // 2],
                    xkv_t.ap()[c * 128:(c + 1) * 128, 0:S // 2])
            dma_w(wq_sb, wq_t, pieces=4)
            # second wave: the upfront qT block's xq columns, then wv for
            # the dripped v-projection, then the rest
            for c in range(KC):
                nc.sync.dma_start(
                    xq_sb[:, c, 0:TT], xq_t.ap()[c * 128:(c + 1) * 128, 0:TT])
            dma_w(wv_sb, wv_t, pieces=4)
            for c in range(KC):
                nc.sync.dma_start(
                    xkv_sb[:, c, S // 2:S],
                    xkv_t.ap()[c * 128:(c + 1) * 128, S // 2:S])
            for c in range(KC):
                nc.sync.dma_start(
                    xq_sb[:, c, TT:T], xq_t.ap()[c * 128:(c + 1) * 128, TT:T])
            if has_bias:
                bq_sb = wpool.tile([128, MC], FP32, tag="bq")
                nc.sync.dma_start(bq_sb[:], bq_t.ap().rearrange("(m p) -> p m", p=128))
                bk_sb = wpool.tile([128, MC], FP32, tag="bk")
                nc.sync.dma_start(bk_sb[:], bk_t.ap().rearrange("(m p) -> p m", p=128))
                bv_sb = wpool.tile([128, MC], FP32, tag="bv")
                nc.sync.dma_start(bv_sb[:], bv_t.ap().rearrange("(m p) -> p m", p=128))
            nc.sync.dma_start(
                wo_sb[:], wo_t.ap().rearrange("(m p) e -> p m e", p=128))

            # ---- kT/qT projections (1024-wide PSUM tiles, DVE drains) ----
            def proj(dst_m, w_sb_, x_sb, m, hh, pool, bias=None):
                # one 512-wide block (the matmul ISA caps the moving dim at
                # 512). Upfront blocks go through psS; blocks dripped into
                # the attention loop use psO so they never block the score
                # tiles' 2-slot psS rotation (which would stall the exps).
                ps = pool.tile([128, TT], FP32,
                               tag="s" if pool is psS else "o")
                for c in range(KC):
                    nc.tensor.matmul(
                        ps[:], w_sb_[:, c, ts(m, 128)],
                        x_sb[:, c, ts(hh, TT)],
                        start=(c == 0), stop=(c == KC - 1))
                dst = dst_m[:, ts(hh, TT)]
                if bias is not None:
                    nc.vector.tensor_scalar_add(dst, ps[:], bias)
                else:
                    nc.vector.tensor_copy(dst, ps[:])

            def emit_proj(which, m, hh, pool=psS):
                if which == 0:
                    proj(kT_sb[:, m, :], wk_sb, xkv_sb, m, hh, pool,
                         bk_sb[:, m:m + 1] if has_bias else None)
                else:
                    proj(qT_sb[:, m, :], wq_sb, xq_sb, m, hh, pool,
                         bq_sb[:, m:m + 1] if has_bias else None)

            # upfront (dense, full-clock after warmup): just the first
            # s-block of kT[m0] and t-block of qT[m0]. kT's later s-blocks
            # (first used at sc 4/8/12) drip through the then-idle psO pool
            # in the first iterations; everything else drips in first-use
            # order (kT/qT m1 at group (0,1) = iter 16+, qT later t-blocks
            # at t-tiles 1-3).
            emit_proj(0, 0, 0)
            emit_proj(0, 0, 1)
            emit_proj(1, 0, 0)
            early_proj = [(0, 0, 2), (0, 0, 3)]
            pending_proj = [(0, 1, 0), (0, 1, 1), (1, 1, 0), (0, 1, 2),
                            (0, 1, 3), (1, 0, 1), (1, 1, 1), (1, 0, 2),
                            (1, 1, 2), (1, 0, 3), (1, 1, 3)]

            # ---- v projection chunk (dripped into the first group) ----
            def emit_vproj(sc):
                ps = psO.tile([128, TT], FP32, tag="o")
                psv = ps[:, 0:DCL]
                last = KC - 1
                for c in range(KC):
                    nc.tensor.matmul(
                        psv, xkv_sb[:, c, ts(sc, 128)], wv_sb[:, c, :],
                        start=(c == 0), stop=(c == last and not has_bias))
                pshc = psv.rearrange("p (h c) -> p h c", c=D_HEAD)
                if has_bias:
                    # bias via DVE add during the copy
                    nc.vector.tensor_scalar_add(
                        v5[:, sc, :, 0, :], pshc, bv_sb[:, 0:1])
                else:
                    nc.vector.tensor_copy(v5[:, sc, :, 0, :], pshc)

            # ---- out-projection chunks (deferred, dripped) ----
            def emit_outproj_chunk(tq, et, on_act=False, pool=None):
                pool = psO if pool is None else pool
                ysb = ypool.tile([128, TT], BF16, tag="y", name=f"ysb{tq}_{et}")
                ps = pool.tile([128, TT], FP32,
                               tag="s" if pool is psS else "o")
                for m in range(MC):
                    nc.tensor.matmul(
                        ps[:], ct_sb[:, m, ts(tq, 128)], wo_sb[:, m, ts(et, TT)],
                        start=(m == 0), stop=(m == MC - 1))
                if on_act:
                    nc.scalar.activation(ysb[:], ps[:], AF.Copy)
                else:
                    nc.vector.tensor_copy(ysb[:], ps[:])
                nc.sync.dma_start(y_t.ap()[ts(tq, 128), ts(et, TT)], ysb[:])

            # ---- attention: QKT/exp stream + lagged PV queue ----
            ctx = {}            # (tt, hp) -> [cA, cB]

            def emit_den(cps, lh, tt, hp, last=False):
                # copy num/den out so the PSUM slot frees after two short
                # copies; the next group's PV reuses it
                den = rpool.tile([64, TT], FP32, tag="den")
                if last:
                    # final group: nothing reuses the PSUM slot and the ACT
                    # engine is idle (exps done) — shortest den->ct latency
                    # gates the tail out-projections
                    nc.scalar.activation(den[:], cps[64:128, :], AF.Copy)
                    rec = rpool.tile([64, TT], FP32, tag="rec")
                    nc.vector.reciprocal_approx_fast(rec[:], den[:])
                    dst = ct_sb[lh:lh + 64, hp, ts(tt, TT)]
                    nc.vector.tensor_mul(dst, cps[0:64, :], rec[:])
                    return
                nc.vector.tensor_copy(den[:], cps[64:128, :])
                num = rpool.tile([64, TT], FP32, tag="num")
                nc.vector.tensor_copy(num[:], cps[0:64, :])
                rec = rpool.tile([64, TT], FP32, tag="rec")
                nc.vector.reciprocal_approx_fast(rec[:], den[:])
                dst = ct_sb[lh:lh + 64, hp, ts(tt, TT)]
                nc.vector.tensor_mul(dst, num[:], rec[:])

            def pop_pv(pvq):
                tt, hp, sc = pvq.pop(0)
                ha, hb = 2 * hp, 2 * hp + 1
                if sc == 0:
                    ctx[(tt, hp)] = [
                        psC.tile([128, TT], FP32, tag="c", name=f"cA{tt}_{hp}"),
                        psC.tile([128, TT], FP32, tag="c", name=f"cB{tt}_{hp}"),
                    ]
                cA, cB = ctx[(tt, hp)]
                start, stop = (sc == 0), (sc == SC - 1)
                nc.tensor.matmul(
                    cA[:], v_sb[:, sc, 2 * ha:2 * ha + 2, :],
                    e_ring[:, sc, 0:TT], start=start, stop=stop)
                nc.tensor.matmul(
                    cB[:], v_sb[:, sc, 2 * hb:2 * hb + 2, :],
                    e_ring[:, sc, TT:2 * TT], start=start, stop=stop)
                if stop:
                    last = (tt == NTT - 1 and hp == HP - 1)
                    emit_den(cA, 0, tt, hp, last)
                    emit_den(cB, 64, tt, hp, last)
                    del ctx[(tt, hp)]

            pvq = []
            deferred = []
            for tt in range(NTT):
                for hp in range(HP):
                    # sc processed in pairs: both QKT pairs (64-partition
                    # split-tile mode) run back-to-back, then both exps, the
                    # 128-wide fillers, and both PV pops — the PE pays its
                    # ~100ns 64<->128 stationary-mode switch once per pair
                    # instead of once per iteration.
                    for sc in range(0, SC, 2):
                        it = hp * SC + sc
                        for d in range(2):
                            scx = sc + d
                            sAB = psS.tile([128, 2 * TT], FP32, tag="s",
                                           name=f"sAB{d}")
                            nc.tensor.matmul(
                                sAB[:, 0:TT], kT_sb[0:64, hp, ts(scx, 128)],
                                qT_sb[0:64, hp, ts(tt, TT)],
                                start=True, stop=True)
                            nc.tensor.matmul(
                                sAB[:, TT:2 * TT],
                                kT_sb[64:128, hp, ts(scx, 128)],
                                qT_sb[64:128, hp, ts(tt, TT)],
                                start=True, stop=True)
                            if has_mask:
                                mt = mpool.tile([128, TT], FP32, tag="m")
                                nc.sync.dma_start(
                                    mt[:], mask_t.ap()[ts(scx, 128), ts(tt, TT)])
                                nc.vector.tensor_add(
                                    sAB[:, 0:TT], sAB[:, 0:TT], mt[:])
                                nc.vector.tensor_add(
                                    sAB[:, TT:2 * TT], sAB[:, TT:2 * TT], mt[:])
                            nc.scalar.activation(e_ring[:, scx, :], sAB[:],
                                                 AF.Exp)
                            pvq.append((tt, hp, scx))
                        # fillers: v-projection chunks, remaining kT/qT
                        # blocks, deferred out-projection chunks
                        if tt == 0 and it < 3 and early_proj:
                            emit_proj(*early_proj.pop(0), pool=psO)
                        if tt == 0 and it + 1 < 3 and early_proj:
                            emit_proj(*early_proj.pop(0), pool=psO)
                        if tt == 0:
                            for itx in (it, it + 1):
                                if 3 <= itx < 3 + SC:
                                    emit_vproj(itx - 3)
                        if tt == 0 and pending_proj:
                            emit_proj(*pending_proj.pop(0))
                        # PV pops first: their den chain gates the psC slot
                        # and ct, so they shouldn't queue behind drains
                        while len(pvq) > 2 * PV_PLAG:
                            pop_pv(pvq)
                        # ct of t-tile tt-1 is final only a few iterations
                        # into (tt, hp0) — keep hp0 drains late, hp1 early.
                        drain = (sc >= 6 if hp == 0 else sc <= 7)
                        if deferred and drain:
                            emit_outproj_chunk(*deferred.pop(0))
                # queue this t-tile's output projection
                for tq in range(tt * (TT // 128), (tt + 1) * (TT // 128)):
                    for et in range(NET):
                        deferred.append((tq, et))
            while pvq:
                pop_pv(pvq)
            # tail out-projections: psS is free once the exps are done, so
            # alternate psO/psS for a 4-deep chunk pipeline
            for i, (tq, et) in enumerate(deferred):
                emit_outproj_chunk(tq, et, on_act=(i % 2 == 0),
                                   pool=(psS if i % 2 else psO))

    nc.compile()
    return nc


class _nullpool:
    def __enter__(self):
        return None

    def __exit__(self, *a):
        return False


def _get_program(has_bias, has_mask):
    key = (has_bias, has_mask)
    if key not in _programs:
        _programs[key] = build_program(has_bias, has_mask)
    return _programs[key]


def kernel(query_states, key_value_states, attention_mask,
           Wq, bq, Wk, bk, Wv, bv, Wo, bo):
    global LAST_EXEC_NS, LAST_RESULTS
    import ml_dtypes
    bf16 = ml_dtypes.bfloat16
    q = np.asarray(query_states, dtype=np.float32)
    kv = np.asarray(key_value_states, dtype=np.float32)
    mask = np.asarray(attention_mask, dtype=np.float32)
    Wq = np.asarray(Wq, np.float32); bq = np.asarray(bq, np.float32)
    Wk = np.asarray(Wk, np.float32); bk = np.asarray(bk, np.float32)
    Wv = np.asarray(Wv, np.float32); bv = np.asarray(bv, np.float32)
    Wo = np.asarray(Wo, np.float32); bo = np.asarray(bo, np.float32)

    has_bias = bool(np.any(bq) or np.any(bk) or np.any(bv))
    has_mask = bool(np.any(mask))
    nc = _get_program(has_bias, has_mask)

    # per-batch activations (shared across the 4 TP cores of each batch)
    xq16 = [np.ascontiguousarray(q[b].T.astype(bf16)) for b in range(DP)]
    xkv16 = [np.ascontiguousarray(kv[b].T.astype(bf16)) for b in range(DP)]

    in_maps = []
    for c in range(N_CORES):
        b, hg = divmod(c, TPG)
        sl = slice(DC * hg, DC * (hg + 1))
        m = {
            "xq_t": xq16[b],
            "xkv_t": xkv16[b],
            "wq_t": np.ascontiguousarray((Wq[sl] * SCALING).T.astype(bf16)),
            "wk_t": np.ascontiguousarray(Wk[sl].T.astype(bf16)),
            "wv_t": np.ascontiguousarray(Wv[sl].T.astype(bf16)),
            "wo_t": np.ascontiguousarray(Wo[:, sl].T.astype(bf16)),
        }
        if has_bias:
            m["bq_t"] = np.ascontiguousarray(bq[sl] * SCALING)
            m["bk_t"] = np.ascontiguousarray(bk[sl])
            m["bv_t"] = np.ascontiguousarray(bv[sl])
        if has_mask:
            mb = np.broadcast_to(mask[b].reshape(-1, mask.shape[-2], mask.shape[-1])[0],
                                 (q.shape[1], kv.shape[1]))
            m["mask_t"] = np.ascontiguousarray(mb.T)
        in_maps.append(m)

    if PROFILE:
        _install_profile_hook()
    res = run_bass_kernel_spmd(nc, in_maps, core_ids=list(range(N_CORES)),
                               trace=bool(PROFILE))
    LAST_EXEC_NS = res.exec_time_ns
    LAST_RESULTS = res
    outs = [res.results[c]["y"].astype(np.float32) for c in range(N_CORES)]
    y = np.stack([sum(outs[b * TPG:(b + 1) * TPG]) for b in range(DP)])
    return (y + bo).astype(np.float32)
